# Initial kernel scaffold
#
"""AttentionFlow Trainium2 Bass kernel.

Math (per batch):
  d = 256; w = [w_c | w_q | w_m]
  s_c[t] = C[t,:] @ w_c ; s_q[j] = Q[j,:] @ w_q ; s_m[t,j] = sum_d C[t,d] w_m[d] Q[j,d]
  sim = s_c[:,None] + s_q[None,:] + s_m + b
  attn = softmax_j(sim)                  (s_c[t] and b cancel -> softmax_j(g), g = s_m + s_q)
  AQ = attn @ Q
  beta = softmax_t(max_j sim)            (b cancels; numerator n[t] = exp(max_j g[t,:]) * exp(s_c[t]))
  AC = beta @ C
  out = concat([C, AQ, C*AQ, C*AC], axis=-1)

Sharding: data-parallel over batch B=32 across 8 NeuronCores (4 batches/core).

On-chip layout per batch (core-local):
  C    [128(t), 8, 256]      natural t-tiles
  ctT  [128(d), 2, 8, 128]   C^T via PE transposes (d-tile k, t-tile i)
  g^T  [128(j), t]           PE matmul, contraction over d, 512-wide chunks
  E^T  = exp(g^T + s_q)      one ACT pass/chunk, per-partition bias = s_q column
  U    [t,257] = E @ [Q|1]   PE, lhsT = E^T slice (no transpose of attn needed)
  AQ   = U[:,0:256] * 1/U[:,256]
  n    = rowmax(E) * exp(s_c)  (rowmax over partitions via gpsimd
                                partition_all_reduce(max) on E^T; s_c row
                                via 1-col-LDW fp32r matmuls + P=1 transposes)
  AC   = (n^T @ C) / (n^T @ 1); broadcast via gpsimd partition_broadcast

Performance structure:
  - The heavy matmuls (sim, AQ, AC) run in float32r (1 cycle/row vs 4 for
    float32 at moving free-dim >= 256; N=1 moving is ISA-illegal for fp32r).
    The BIR verifier requires fp32r matmul operands to be *written rounded*
    by a compute instruction (DMA-written buffers won't do), so ACT/DVE
    copies produce rounded qwT/ctT/ET/Qr/Cr/n.
  - The beta tail of batch b (n transposes, AC, broadcast, out4, output
    DMAs) is software-pipelined into batch b+1's body so the PE stream
    never stalls on the GpSimd reduce chain.
  - Outputs are staged in per-batch tiles and written with a few large
    DMAs (SP descriptor dispatch is ~0.7us per dma_start; per-tile DMAs
    were the original bottleneck). out[:, :, 0:256] is DMA'd straight
    from the C tile.
"""

import numpy as np

import concourse.bass as bass
import concourse.mybir as mybir
import concourse.tile as tile
from concourse import bacc
from concourse import bass_isa
from concourse.bass_utils import run_bass_kernel_spmd
from concourse.masks import make_identity

F32 = mybir.dt.float32
F32R = mybir.dt.float32r
AF = mybir.ActivationFunctionType
ALU = mybir.AluOpType
AX = mybir.AxisListType

B, T, J, D = 32, 1024, 128, 256
NCORES = 8
BPC = B // NCORES      # batches per core
NT = T // 128          # t-tiles per batch
ND = D // 128          # d-tiles

# float32r for the two big matmuls; set False for an all-fp32 fallback.
USE_F32R = True


def _rr(ap):
    """float32r view of an f32 AP (for rounded producers + matmul operands)."""
    return ap.bitcast(F32R) if USE_F32R else ap


def _bcast_row(ap_1d, nparts):
    """DRAM AP [n] -> [nparts, n] with partition stride 0 (DMA broadcast)."""
    return bass.AP(
        tensor=ap_1d.tensor, offset=ap_1d.offset, ap=[[0, nparts]] + list(ap_1d.ap)
    )


def build_nc(use_f32r=None):
    global USE_F32R
    if use_f32r is not None:
        USE_F32R = use_f32r
    nc = bacc.Bacc()
    ctx_in = nc.declare_dram_parameter("context", [BPC, T, D], F32, isOutput=False)
    qry_in = nc.declare_dram_parameter("query", [BPC, J, D], F32, isOutput=False)
    w_in = nc.declare_dram_parameter("w", [3 * D], F32, isOutput=False)
    out_ext = nc.declare_dram_parameter("out", [BPC, T, 4 * D], F32, isOutput=True)

    with tile.TileContext(nc) as tc:
        _body(tc, ctx_in, qry_in, w_in, out_ext)
    nc.finalize()
    return nc


def _body(tc, ctx_in, qry_in, w_in, out_ext):
    nc = tc.nc
    from contextlib import ExitStack

    with ExitStack() as ctx:
        consts = ctx.enter_context(tc.tile_pool(name="consts", bufs=1))
        big = ctx.enter_context(tc.tile_pool(name="big", bufs=2))
        work = ctx.enter_context(tc.tile_pool(name="work", bufs=4))
        batch = ctx.enter_context(tc.tile_pool(name="batch", bufs=2))
        # PSUM budget (8 banks): g 2 + ct 2 + u 2 + s 1 + ac 1 = 8
        ps_g = ctx.enter_context(tc.tile_pool(name="ps_g", bufs=1, space="PSUM"))
        ps_ct = ctx.enter_context(tc.tile_pool(name="ps_ct", bufs=2, space="PSUM"))
        ps_u = ctx.enter_context(tc.tile_pool(name="ps_u", bufs=3, space="PSUM"))
        ps_s = ctx.enter_context(tc.tile_pool(name="ps_s", bufs=1, space="PSUM"))
        ps_ac = ctx.enter_context(tc.tile_pool(name="ps_ac", bufs=1, space="PSUM"))

        # --- constants (identity first: it gates the first PE transposes) ---
        ident = consts.tile([128, 128], F32)
        make_identity(nc, ident)
        ones_col = consts.tile([128, 1], F32)
        nc.vector.memset(ones_col, 1.0)
        ones_row = consts.tile([1, 128], F32)
        nc.vector.memset(ones_row, 1.0)
        ones2 = consts.tile([128, 2], F32)
        nc.vector.memset(ones2, 1.0)
        ones2_r = consts.tile([128, 2], F32)
        nc.scalar.copy(_rr(ones2_r), ones2)
        ones_row_r = consts.tile([1, 128], F32)
        nc.scalar.copy(_rr(ones_row_r), ones_row)

        # w_c / w_m as per-partition columns (two d-tiles each)
        wc_raw = consts.tile([128, ND], F32)
        wm_cols = consts.tile([128, ND], F32)
        for k in range(ND):
            nc.gpsimd.dma_start(
                out=wc_raw[:, k : k + 1],
                in_=w_in[k * 128 : (k + 1) * 128].rearrange("(p o) -> p o", o=1),
            )
            nc.gpsimd.dma_start(
                out=wm_cols[:, k : k + 1],
                in_=w_in[2 * D + k * 128 : 2 * D + (k + 1) * 128].rearrange(
                    "(p o) -> p o", o=1
                ),
            )
        # rounded copy so the s_c row matmuls can run in fp32r
        wc_cols = consts.tile([128, ND], F32)
        nc.scalar.copy(_rr(wc_cols), wc_raw)
        # w_q broadcast to all partitions (for s_q = rowsum(Q * w_q))
        wq_b = consts.tile([128, D], F32)
        nc.gpsimd.dma_start(out=wq_b, in_=_bcast_row(w_in[D : 2 * D], 128))

        def beta_tail(S):
            """Finish batch S (the beta path + o4 + output DMAs). Runs inside
            the NEXT batch body so the PE stream never stalls on it."""
            b, C, Cr, n_rows, aqo3, o4_all = S
            n_ps = ps_s.tile([128, NT], F32, tag="s")
            for i in range(NT):
                nr = n_rows[i // 4]
                nc.tensor.transpose(
                    n_ps[:, i : i + 1],
                    nr[0:1, (i % 4) * 128 : (i % 4 + 1) * 128],
                    ident[0:1, 0:1],
                )
            n_all = batch.tile([128, NT], F32, tag="n_all")
            nc.vector.tensor_copy(_rr(n_all), n_ps)

            ac_ps = ps_ac.tile([1, 256], F32, tag="ac")
            for i in range(NT):
                nc.tensor.matmul(
                    ac_ps,
                    lhsT=_rr(n_all[:, i : i + 1]),
                    rhs=_rr(Cr[:, i, :]),
                    start=(i == 0),
                    stop=(i == NT - 1),
                )
            s_ps = ps_s.tile([1, NT], F32, tag="s")
            nc.tensor.matmul(s_ps, lhsT=_rr(ones2_r[:, 0:1]), rhs=_rr(n_all))
            s_tot = batch.tile([1, 1], F32, tag="s_tot")
            nc.vector.reduce_sum(out=s_tot, in_=s_ps, axis=AX.X)
            r_s = batch.tile([1, 1], F32, tag="r_s")
            nc.vector.reciprocal(r_s, s_tot)
            ac_row = batch.tile([1, 256], F32, tag="ac_row")
            nc.scalar.activation(ac_row, ac_ps, AF.Copy, scale=r_s)

            acb = batch.tile([128, 256], F32, tag="acb")
            nc.gpsimd.partition_broadcast(acb, ac_row, channels=128)

            for i in range(NT):
                nc.vector.tensor_mul(o4_all[:, i, :], C[:, i, :], acb)
            out_r = out_ext[b].rearrange("(i p) d -> p i d", p=128)
            h = NT // 2
            nc.sync.dma_start(out=out_r[:, 0:h, 256:768], in_=aqo3[:, 0:h, :])
            nc.sync.dma_start(out=out_r[:, h:NT, 256:768], in_=aqo3[:, h:NT, :])
            nc.sync.dma_start(out=out_r[:, 0:h, 768:1024], in_=o4_all[:, 0:h, :])
            nc.sync.dma_start(out=out_r[:, h:NT, 768:1024], in_=o4_all[:, h:NT, :])

        prev = None
        for b in range(BPC):
            # ---- loads ----
            Q = batch.tile([128, D], F32, tag="Q")
            nc.sync.dma_start(out=Q, in_=qry_in[b])
            C = big.tile([128, NT, D], F32, tag="C")
            ctx_r = ctx_in[b].rearrange("(i p) d -> p i d", p=128)
            nc.sync.dma_start(out=C[:, 0 : NT // 2, :], in_=ctx_r[:, 0 : NT // 2, :])
            nc.sync.dma_start(out=C[:, NT // 2 : NT, :], in_=ctx_r[:, NT // 2 : NT, :])
            # out component 1 = context passthrough, straight from the C tile
            nc.sync.dma_start(
                out=out_ext[b].rearrange("(i p) d -> p i d", p=128)[:, :, 0:256],
                in_=C,
            )
            # rounded copy of Q for the fp32r AQ matmul
            Qr = batch.tile([128, D], F32, tag="Qr")
            nc.scalar.copy(_rr(Qr), Q)
            # rounded copy of C for the fp32r beta-weighted sum (split ACT/DVE)
            Cr = big.tile([128, NT, D], F32, tag="Cr")
            nc.scalar.copy(_rr(Cr[:, 0 : NT // 2, :]), C[:, 0 : NT // 2, :])
            nc.vector.tensor_copy(_rr(Cr[:, NT // 2 : NT, :]), C[:, NT // 2 : NT, :])

            # ---- Q^T, with w_m folded in: qwT[d, j] = Q[j, d] * w_m[d] ----
            qt_ps = ps_ct.tile([128, D], F32, tag="ct")
            for k in range(ND):
                nc.tensor.transpose(
                    qt_ps[:, k * 128 : (k + 1) * 128], Q[:, k * 128 : (k + 1) * 128], ident
                )
            qwT = batch.tile([128, D], F32, tag="qwT")
            for k in range(ND):
                nc.scalar.activation(
                    _rr(qwT[:, k * 128 : (k + 1) * 128]),
                    qt_ps[:, k * 128 : (k + 1) * 128],
                    AF.Copy,
                    scale=wm_cols[:, k : k + 1],
                )

            # ---- s_q column: rowsum(Q * w_q) ----
            sq_scr = batch.tile([128, D], F32, tag="sq_scr")
            sq_col = batch.tile([128, 1], F32, tag="sq_col")
            nc.vector.tensor_mul(sq_scr, Q, wq_b)
            nc.vector.reduce_sum(out=sq_col, in_=sq_scr, axis=AX.X)

            # ---- finish PREVIOUS batch early (its inputs are all ready) ----
            if prev is not None:
                beta_tail(prev)

            # ---- C^T via PE transposes ----
            ctT = big.tile([128, ND, NT, 128], F32, tag="ctT")
            for i2 in range(NT // 2):
                ct_ps = ps_ct.tile([128, 2 * ND * 128], F32, tag="ct")
                for u in range(2):
                    i = 2 * i2 + u
                    for k in range(ND):
                        nc.tensor.transpose(
                            ct_ps[:, (2 * u + k) * 128 : (2 * u + k + 1) * 128],
                            C[:, i, k * 128 : (k + 1) * 128],
                            ident,
                        )
                dst = _rr(ctT[:, :, 2 * i2 : 2 * i2 + 2, :])
                srcv = ct_ps.rearrange("p (t k x) -> p k t x", t=2, k=ND)
                if i2 % 2 == 0:
                    nc.scalar.copy(dst, srcv)
                else:
                    nc.vector.tensor_copy(dst, srcv)

            # ---- g^T = (Q*w_m) @ C^T and s_c^T = w_c^T @ C^T per 512-chunk ----
            ET = big.tile([128, T], F32, tag="ET")
            expsc_row = batch.tile([1, T], F32, tag="expsc_row")
            for c in range(T // 512):
                g_ps = ps_g.tile([128, 512], F32, tag="g")
                scr_ps = ps_u.tile([1, 512], F32, tag="u")
                for k in range(ND):
                    nc.tensor.matmul(
                        g_ps,
                        lhsT=_rr(qwT[:, k * 128 : (k + 1) * 128]),
                        rhs=_rr(ctT[:, k, 4 * c : 4 * (c + 1), :]),
                        start=(k == 0),
                        stop=(k == ND - 1),
                    )
                for k in range(ND):
                    nc.tensor.matmul(
                        scr_ps,
                        lhsT=_rr(wc_cols[:, k : k + 1]),
                        rhs=_rr(ctT[:, k, 4 * c : 4 * (c + 1), :]),
                        start=(k == 0),
                        stop=(k == ND - 1),
                    )
                nc.scalar.activation(
                    _rr(ET[:, c * 512 : (c + 1) * 512]), g_ps, AF.Exp, bias=sq_col
                )
                nc.scalar.activation(
                    expsc_row[:, c * 512 : (c + 1) * 512], scr_ps, AF.Exp
                )

            # ---- rowmax over j via GpSimd cross-partition max on E^T ----
            me_all = big.tile([128, T], F32, tag="me")
            n_rows = []
            for c in range(T // 512):
                nc.gpsimd.partition_all_reduce(
                    me_all[:, c * 512 : (c + 1) * 512],
                    ET[:, c * 512 : (c + 1) * 512],
                    channels=128,
                    reduce_op=bass_isa.ReduceOp.max,
                )
                nr = batch.tile([1, 512], F32, tag=f"n_row{c}")
                nc.vector.tensor_mul(
                    nr,
                    me_all[0:1, c * 512 : (c + 1) * 512],
                    expsc_row[:, c * 512 : (c + 1) * 512],
                )
                n_rows.append(nr)

            # ---- per t-tile: U = E @ [Q|1], AQ, batch staging writes ----
            aqo3 = big.tile([128, NT, 2 * D], F32, tag="aqo3")
            o4_all = big.tile([128, NT, D], F32, tag="o4")
            for i in range(NT):
                et_sl = ET[:, i * 128 : (i + 1) * 128]

                u_ps = ps_u.tile([128, 258], F32, tag="u")
                nc.tensor.matmul(u_ps[:, 0:256], lhsT=_rr(et_sl), rhs=_rr(Qr))
                nc.tensor.matmul(u_ps[:, 256:258], lhsT=_rr(et_sl), rhs=_rr(ones2_r))

                r_col = work.tile([128, 1], F32, tag="r_col")
                nc.vector.reciprocal(r_col, u_ps[:, 256:257])
                if i % 2 == 0:
                    nc.scalar.activation(
                        aqo3[:, i, 0:256], u_ps[:, 0:256], AF.Copy, scale=r_col
                    )
                else:
                    r_b = bass.AP(
                        tensor=r_col.tensor, offset=r_col.offset,
                        ap=[list(r_col.ap)[0], [0, 256]],
                    )
                    nc.vector.tensor_mul(aqo3[:, i, 0:256], u_ps[:, 0:256], r_b)
                nc.vector.tensor_mul(aqo3[:, i, 256:512], aqo3[:, i, 0:256], C[:, i, :])

            prev = (b, C, Cr, n_rows, aqo3, o4_all)

        beta_tail(prev)


_NC_CACHE = {}


def kernel(context, query, w, b, _trace=False):
    context = np.ascontiguousarray(context, dtype=np.float32)
    query = np.ascontiguousarray(query, dtype=np.float32)
    w = np.ascontiguousarray(w, dtype=np.float32)

    if "nc" not in _NC_CACHE:
        _NC_CACHE["nc"] = build_nc()
    nc = _NC_CACHE["nc"]

    in_maps = [
        {
            "context": context[i * BPC : (i + 1) * BPC],
            "query": query[i * BPC : (i + 1) * BPC],
            "w": w,
        }
        for i in range(NCORES)
    ]
    try:
        res = run_bass_kernel_spmd(
            nc, in_maps, core_ids=list(range(NCORES)), trace=_trace
        )
    except Exception:
        # A previous process may have left the device wedged; reset and retry.
        import ctypes

        import jax

        jax.devices()
        lib = ctypes.CDLL("/opt/axon/libaxon_pjrt.so")
        if hasattr(lib, "axon_reset"):
            lib.axon_reset()
        res = run_bass_kernel_spmd(
            nc, in_maps, core_ids=list(range(NCORES)), trace=_trace
        )
    out = np.concatenate([res.results[i]["out"] for i in range(NCORES)], axis=0)
    if _trace:
        kernel.last_exec_time_ns = res.exec_time_ns
        kernel.last_results = res
    return out


if __name__ == "__main__":
    rng = np.random.default_rng(0)
    inputs = {
        "context": rng.standard_normal((B, T, D), dtype=np.float32),
        "query": rng.standard_normal((B, J, D), dtype=np.float32),
        "w": (rng.standard_normal(3 * D).astype(np.float32) / np.sqrt(3 * D)),
        "b": np.zeros(1, np.float32),
    }
    out = kernel(**inputs)
    print("out", out.shape, out.dtype, float(np.abs(out).mean()))



# revision 12
# speedup vs baseline: 1.0432x; 1.0432x over previous
"""AttentionFlow Trainium2 Bass kernel (v2).

Math (per batch):
  d = 256; w = [w_c | w_q | w_m]
  sim[t,j] = s_c[t] + s_q[j] + sum_d C[t,d] w_m[d] Q[j,d]   (+b, which cancels)
  attn = softmax_j(sim);  AQ = attn @ Q
  beta = softmax_t(max_j sim);  AC = beta @ C
  out = concat([C, AQ, C*AQ, C*AC], axis=-1)

Sharding: data-parallel over batch B=32 across 8 NeuronCores (4 batches/core).

v2 design (vs v1):
  - Permuted t-layout t = 8p + i (partition-major): the whole out row block of
    a partition is contiguous in HBM, so each batch's output is ONE DMA with
    128 x 32KB descriptors instead of ~3k 1-2KB lines.  All per-t math is
    permutation-invariant; only the T-sums (AC, s_tot) mix t and they are
    order-free.
  - Everything is staged in one [128, NT, 4D] tile per batch
    (cols 0:256 = C landed by the input DMA, 256:512 aq, 512:768 o3=C*aq,
    768:1024 o4=C*AC).
  - E' = exp(g + s_q + s_c) holds the FULL similarity: s_q enters as the ACT
    bias column of the exp, s_c enters via a rank-1 PE matmul
    (ones_row^T x sc_row) accumulated into the g PSUM bank.  Then
    n[t] = colmax_j E' directly (no exp(s_c) row pass, no n-row muls), and
    the attn normalization is unchanged (the exp(s_c[t]) factor cancels in
    U[:,0:256]/U[:,256]).
  - n columns come from GpSimd partition_all_reduce(max) + a diagonal
    extract (gpsimd mul by a replicated identity, DVE reduce_max) -- no
    more per-128 N=1 PE transposes of the n row.
  - U = E' @ [Q | 1] as ONE N=257 fp32r matmul per t-tile (ones column baked
    into Qaug); AC = n^T @ [C | 1] with the ones column baked into Cr, so
    s_tot falls out of the same accumulation (no separate sum matmul).
  - AC is accumulated eagerly per 512-chunk so only a short tail remains
    after the last chunk; the tail of batch b is issued inside batch b+1.
  - Elementwise work is spread: ACT (exp, aq-even, copies), DVE (aq-odd, o4,
    reciprocals, diag reduce), GpSimd (all_reduce, diag mul, o3, broadcast).
  - C of batch b+1 is prefetched during batch b (stage pool bufs=3).
"""

import numpy as np

import concourse.bass as bass
import concourse.mybir as mybir
import concourse.tile as tile
from concourse import bacc
from concourse import bass_isa
from concourse.bass_utils import run_bass_kernel_spmd
from concourse.masks import make_identity

F32 = mybir.dt.float32
F32R = mybir.dt.float32r
AF = mybir.ActivationFunctionType
ALU = mybir.AluOpType
AX = mybir.AxisListType

B, T, J, D = 32, 1024, 128, 256
NCORES = 8
BPC = B // NCORES      # batches per core
NT = T // 128          # t-tiles per batch
ND = D // 128          # d-tiles
NCH = T // 512         # 512-wide chunks per batch
TPC = 4                # t-tiles per chunk

USE_F32R = True


def _rr(ap):
    """float32r view of an f32 AP (for rounded producers + matmul operands)."""
    return ap.bitcast(F32R) if USE_F32R else ap


def _bcast_row(ap_1d, nparts):
    """DRAM AP [n] -> [nparts, n] with partition stride 0 (DMA broadcast)."""
    return bass.AP(
        tensor=ap_1d.tensor, offset=ap_1d.offset, ap=[[0, nparts]] + list(ap_1d.ap)
    )


def _fbcast(ap_col, n):
    """[128,1] column AP -> [128, n] with free stride 0."""
    return bass.AP(
        tensor=ap_col.tensor, offset=ap_col.offset,
        ap=[list(ap_col.ap)[0], [0, n]],
    )


def _tile_bcast(ap_2d, reps):
    """[128, n] AP -> [128, reps, n] with 0-stride middle dim."""
    a = list(ap_2d.ap)
    return bass.AP(
        tensor=ap_2d.tensor, offset=ap_2d.offset,
        ap=[a[0], [0, reps]] + a[1:],
    )


def build_nc(use_f32r=None):
    global USE_F32R
    if use_f32r is not None:
        USE_F32R = use_f32r
    nc = bacc.Bacc()
    ctx_in = nc.declare_dram_parameter("context", [BPC, T, D], F32, isOutput=False)
    qry_in = nc.declare_dram_parameter("query", [BPC, J, D], F32, isOutput=False)
    w_in = nc.declare_dram_parameter("w", [3 * D], F32, isOutput=False)
    out_ext = nc.declare_dram_parameter("out", [BPC, T, 4 * D], F32, isOutput=True)

    with tile.TileContext(nc) as tc:
        _body(tc, ctx_in, qry_in, w_in, out_ext)
    nc.finalize()
    return nc


def _body(tc, ctx_in, qry_in, w_in, out_ext):
    nc = tc.nc
    from contextlib import ExitStack

    with ExitStack() as ctx:
        consts = ctx.enter_context(tc.tile_pool(name="consts", bufs=1))
        stage_p = ctx.enter_context(tc.tile_pool(name="stage", bufs=4))
        big = ctx.enter_context(tc.tile_pool(name="big", bufs=2))
        work = ctx.enter_context(tc.tile_pool(name="work", bufs=2))
        tmp = ctx.enter_context(tc.tile_pool(name="tmp", bufs=1))
        # PSUM budget (8 banks): tr 2 + g 2 + sc 1 + u 2 + ac 1 = 8
        ps_tr = ctx.enter_context(tc.tile_pool(name="ps_tr", bufs=2, space="PSUM"))
        ps_g = ctx.enter_context(tc.tile_pool(name="ps_g", bufs=2, space="PSUM"))
        ps_sc = ctx.enter_context(tc.tile_pool(name="ps_sc", bufs=1, space="PSUM"))
        ps_u = ctx.enter_context(tc.tile_pool(name="ps_u", bufs=2, space="PSUM"))
        ps_ac = ctx.enter_context(tc.tile_pool(name="ps_ac", bufs=1, space="PSUM"))

        loads = {}

        def load_batch(bb, nsplit):
            st = stage_p.tile([128, NT, 4 * D], F32, tag="stage")
            qt = work.tile([128, D], F32, tag="Q")
            src = ctx_in[bb].rearrange("(p i) d -> p i d", i=NT)
            step = NT // nsplit
            for s in range(nsplit):
                nc.sync.dma_start(
                    out=st[:, s * step : (s + 1) * step, 0:D],
                    in_=src[:, s * step : (s + 1) * step, :],
                )
            nc.sync.dma_start(out=qt, in_=qry_in[bb])
            loads[bb] = (st, qt)

        # batch-0 input DMAs dispatched before all the consts traffic (v7)
        load_batch(0, 4)

        # --- constants (identity first: it gates the first PE transposes) ---
        ident = consts.tile([128, 128], F32)
        make_identity(nc, ident)
        ident4 = consts.tile([128, TPC, 128], F32)
        for j in range(TPC):
            nc.scalar.copy(ident4[:, j, :], ident)
        ones_col = consts.tile([128, 1], F32)
        nc.vector.memset(ones_col, 1.0)
        ones_row = consts.tile([1, 128], F32)
        nc.vector.memset(ones_row, 1.0)
        ones_row_r = consts.tile([1, 128], F32)
        nc.scalar.copy(_rr(ones_row_r), ones_row)

        # w_c / w_m as per-partition columns (two d-tiles each)
        wc_raw = consts.tile([128, ND], F32)
        wm_cols = consts.tile([128, ND], F32)
        for k in range(ND):
            nc.gpsimd.dma_start(
                out=wc_raw[:, k : k + 1],
                in_=w_in[k * 128 : (k + 1) * 128].rearrange("(p o) -> p o", o=1),
            )
            nc.gpsimd.dma_start(
                out=wm_cols[:, k : k + 1],
                in_=w_in[2 * D + k * 128 : 2 * D + (k + 1) * 128].rearrange(
                    "(p o) -> p o", o=1
                ),
            )
        # rounded copy so the s_c row matmuls can run in fp32r
        wc_cols = consts.tile([128, ND], F32)
        nc.scalar.copy(_rr(wc_cols), wc_raw)
        # w_q broadcast to all partitions (for s_q = rowsum(Q * w_q))
        wq_b = consts.tile([128, D], F32)
        nc.gpsimd.dma_start(out=wq_b, in_=_bcast_row(w_in[D : 2 * D], 128))

        def beta_tail_head(S):
            """Deferred AC matmuls + s_tot -> ac_row -> broadcast for batch
            S, issued after the NEXT batch's transposes so the n_all chain
            has a full block of PE work as cover (v7)."""
            b, st, n_all, Cr = S
            ac_ps = ps_ac.tile([1, D + 2], F32, tag="ac")
            for ii in range(NT):
                nc.tensor.matmul(
                    ac_ps,
                    lhsT=_rr(n_all[:, ii : ii + 1]),
                    rhs=_rr(Cr[:, ii, :]),
                    start=(ii == 0),
                    stop=(ii == NT - 1),
                )
            r_s = work.tile([1, 1], F32, tag="r_s")
            nc.vector.reciprocal(r_s, ac_ps[0:1, D : D + 1])
            ac_row = work.tile([1, D], F32, tag="ac_row")
            nc.scalar.activation(ac_row, ac_ps[0:1, 0:D], AF.Copy, scale=r_s)
            acb = work.tile([128, D], F32, tag="acb")
            nc.gpsimd.partition_broadcast(acb, ac_row, channels=128)
            return acb

        def beta_tail_finish(S, acb, last=False):
            """o4 = C * AC (DVE) + staged output DMA, in i-halves.  For the
            last batch the C|aq|o3 columns go out as soon as o3 is done and
            only the 1MB o4 column block trails the beta chain (v7)."""
            b, st = S[0], S[1]
            out_r = out_ext[b].rearrange("(p i) d -> p i d", i=NT)
            h = NT // 2
            if last:
                nc.sync.dma_start(
                    out=out_r[:, :, 0 : 3 * D], in_=st[:, :, 0 : 3 * D]
                )
            for s in range(2):
                sl = slice(s * h, (s + 1) * h)
                nc.vector.tensor_mul(
                    st[:, sl, 3 * D : 4 * D], st[:, sl, 0:D], _tile_bcast(acb, h)
                )
                if last:
                    nc.sync.dma_start(
                        out=out_r[:, sl, 3 * D : 4 * D],
                        in_=st[:, sl, 3 * D : 4 * D],
                    )
                else:
                    nc.sync.dma_start(out=out_r[:, sl, :], in_=st[:, sl, :])

        prev = None
        for b in range(BPC):
            if b + 1 < BPC:
                load_batch(b + 1, 2)
            st, Q = loads.pop(b)

            # ---- Q^T, with w_m folded in: qwT[d, j] = Q[j, d] * w_m[d] ----
            qt_ps = ps_tr.tile([128, D], F32, tag="tr")
            for k in range(ND):
                nc.tensor.transpose(
                    qt_ps[:, k * 128 : (k + 1) * 128], Q[:, k * 128 : (k + 1) * 128],
                    ident,
                )
            qwT = work.tile([128, D], F32, tag="qwT")
            for k in range(ND):
                nc.scalar.activation(
                    _rr(qwT[:, k * 128 : (k + 1) * 128]),
                    qt_ps[:, k * 128 : (k + 1) * 128],
                    AF.Copy,
                    scale=wm_cols[:, k : k + 1],
                )

            # ---- Qaug = [Q | 1] rounded (rhs of the U matmuls) ----
            Qaug = work.tile([128, D + 2], F32, tag="Qaug")
            nc.scalar.copy(_rr(Qaug[:, 0:D]), Q)
            nc.vector.tensor_copy(
                _rr(Qaug[:, D : D + 2]), _fbcast(ones_col, 2)
            )

            # ---- s_q column: rowsum(Q * w_q) ----
            sq_scr = tmp.tile([128, D], F32, tag="sq_scr")
            sq_col = work.tile([128, 1], F32, tag="sq_col")
            nc.vector.tensor_mul(sq_scr, Q, wq_b)
            nc.vector.reduce_sum(out=sq_col, in_=sq_scr, axis=AX.X)

            # ---- C^T via PE transposes ----
            ctT = big.tile([128, ND, NT, 128], F32, tag="ctT")
            for i2 in range(NT // 2):
                ct_ps = ps_tr.tile([128, 2 * ND * 128], F32, tag="tr")
                for u in range(2):
                    i = 2 * i2 + u
                    for k in range(ND):
                        nc.tensor.transpose(
                            ct_ps[:, (2 * u + k) * 128 : (2 * u + k + 1) * 128],
                            st[:, i, k * 128 : (k + 1) * 128],
                            ident,
                        )
                dst = _rr(ctT[:, :, 2 * i2 : 2 * i2 + 2, :])
                srcv = ct_ps.rearrange("p (t k x) -> p k t x", t=2, k=ND)
                if i2 % 2 == 0:
                    nc.scalar.copy(dst, srcv)
                else:
                    nc.vector.tensor_copy(dst, srcv)

            # ---- previous batch: deferred AC + s_tot/ac_row/bcast (v7) ----
            acb_prev = beta_tail_head(prev) if prev is not None else None

            # ---- per 512-chunk: g + s_c row matmuls ----
            ET = big.tile([128, T], F32, tag="ET")
            n_all = work.tile([128, NT], F32, tag="n_all")
            g_list = []
            for c in range(NCH):
                g_ps = ps_g.tile([128, 512], F32, tag="g")
                scp = ps_sc.tile([1, 512], F32, tag="sc")
                for k in range(ND):
                    nc.tensor.matmul(
                        g_ps,
                        lhsT=_rr(qwT[:, k * 128 : (k + 1) * 128]),
                        rhs=_rr(ctT[:, k, TPC * c : TPC * (c + 1), :]),
                        start=(k == 0),
                        stop=False,
                        skip_group_check=True,
                    )
                for k in range(ND):
                    nc.tensor.matmul(
                        scp,
                        lhsT=_rr(wc_cols[:, k : k + 1]),
                        rhs=_rr(ctT[:, k, TPC * c : TPC * (c + 1), :]),
                        start=(k == 0),
                        stop=(k == ND - 1),
                    )
                sc_row = tmp.tile([1, 512], F32, tag=f"sc_row{c}")
                nc.vector.tensor_copy(_rr(sc_row), scp)
                g_list.append((g_ps, sc_row))

            # ---- Cr = [C | 1] rounded (rhs of the AC matmuls) ----
            Cr = big.tile([128, NT, D + 2], F32, tag="Cr")
            nc.vector.tensor_copy(
                _rr(Cr[:, :, D : D + 2]),
                bass.AP(tensor=ones_col.tensor, offset=ones_col.offset,
                        ap=[list(ones_col.ap)[0], [0, NT], [0, 2]]),
            )
            h = NT // 2
            nc.scalar.copy(_rr(Cr[:, 0:h, 0:D]), st[:, 0:h, 0:D])
            nc.vector.tensor_copy(_rr(Cr[:, h:NT, 0:D]), st[:, h:NT, 0:D])

            # ---- per chunk: fold s_c (rank-1), E' = exp, colmax, diag ----
            # o4 + out-DMA of the previous batch are issued between the two
            # chunks so the DVE has work while the GpSimd all_reduce runs.
            me_list = []
            for c in range(NCH):
                g_ps, sc_row = g_list[c]
                nc.tensor.matmul(
                    g_ps,
                    lhsT=_rr(ones_row_r),
                    rhs=_rr(sc_row),
                    start=False,
                    stop=True,
                    skip_group_check=True,
                )
                nc.scalar.activation(
                    _rr(ET[:, c * 512 : (c + 1) * 512]), g_ps, AF.Exp, bias=sq_col
                )
                me = tmp.tile([128, 512], F32, tag=f"me{c}")
                nc.gpsimd.partition_all_reduce(
                    me,
                    ET[:, c * 512 : (c + 1) * 512],
                    channels=128,
                    reduce_op=bass_isa.ReduceOp.max,
                )
                me_list.append(me)
            for c in range(NCH):
                me = me_list[c]
                scr = tmp.tile([128, TPC, 128], F32, tag=f"scr{c}")
                nc.vector.tensor_mul(
                    scr, me.rearrange("p (i r) -> p i r", r=128), ident4
                )
                nc.vector.reduce_max(
                    out=_rr(n_all[:, TPC * c : TPC * (c + 1)]), in_=scr, axis=AX.X
                )

            # ---- per t-tile: U = E' @ [Q|1]; aq on ACT; o3 on DVE ----
            r_all = work.tile([128, NT], F32, tag="r_all")
            for i in range(NT):
                u_ps = ps_u.tile([128, D + 2], F32, tag="u")
                nc.tensor.matmul(
                    u_ps, lhsT=_rr(ET[:, i * 128 : (i + 1) * 128]), rhs=_rr(Qaug)
                )
                nc.vector.reciprocal(r_all[:, i : i + 1], u_ps[:, D : D + 1])
                nc.scalar.activation(
                    st[:, i, D : 2 * D], u_ps[:, 0:D], AF.Copy,
                    scale=r_all[:, i : i + 1],
                )
                if i % TPC == TPC - 1:
                    # o3 for this chunk's 4 tiles (one DVE pass)
                    j0 = i - (TPC - 1)
                    nc.vector.tensor_mul(
                        st[:, j0 : i + 1, 2 * D : 3 * D],
                        st[:, j0 : i + 1, D : 2 * D],
                        st[:, j0 : i + 1, 0:D],
                    )

            # ---- previous batch: o4 + output DMA (after the U loop so the
            #      DVE recips/aq aren't stuck behind the big o4 pass) ----
            if prev is not None:
                beta_tail_finish(prev, acb_prev)
            prev = (b, st, n_all, Cr)

        beta_tail_finish(prev, beta_tail_head(prev), last=True)


_NC_CACHE = {}


def kernel(context, query, w, b, _trace=False):
    context = np.ascontiguousarray(context, dtype=np.float32)
    query = np.ascontiguousarray(query, dtype=np.float32)
    w = np.ascontiguousarray(w, dtype=np.float32)

    if "nc" not in _NC_CACHE:
        _NC_CACHE["nc"] = build_nc()
    nc = _NC_CACHE["nc"]

    in_maps = [
        {
            "context": context[i * BPC : (i + 1) * BPC],
            "query": query[i * BPC : (i + 1) * BPC],
            "w": w,
        }
        for i in range(NCORES)
    ]
    try:
        res = run_bass_kernel_spmd(
            nc, in_maps, core_ids=list(range(NCORES)), trace=_trace
        )
    except Exception:
        # A previous process may have left the device wedged; reset and retry.
        import ctypes

        import jax

        jax.devices()
        lib = ctypes.CDLL("/opt/axon/libaxon_pjrt.so")
        if hasattr(lib, "axon_reset"):
            lib.axon_reset()
        res = run_bass_kernel_spmd(
            nc, in_maps, core_ids=list(range(NCORES)), trace=_trace
        )
    out = np.concatenate([res.results[i]["out"] for i in range(NCORES)], axis=0)
    if _trace:
        kernel.last_exec_time_ns = res.exec_time_ns
        kernel.last_results = res
    return out


if __name__ == "__main__":
    rng = np.random.default_rng(0)
    inputs = {
        "context": rng.standard_normal((B, T, D), dtype=np.float32),
        "query": rng.standard_normal((B, J, D), dtype=np.float32),
        "w": (rng.standard_normal(3 * D).astype(np.float32) / np.sqrt(3 * D)),
        "b": np.zeros(1, np.float32),
    }
    out = kernel(**inputs)
    print("out", out.shape, out.dtype, float(np.abs(out).mean()))


# revision 13
# speedup vs baseline: 1.0969x; 1.0514x over previous
"""AttentionFlow Trainium2 Bass kernel (v2).

Math (per batch):
  d = 256; w = [w_c | w_q | w_m]
  sim[t,j] = s_c[t] + s_q[j] + sum_d C[t,d] w_m[d] Q[j,d]   (+b, which cancels)
  attn = softmax_j(sim);  AQ = attn @ Q
  beta = softmax_t(max_j sim);  AC = beta @ C
  out = concat([C, AQ, C*AQ, C*AC], axis=-1)

Sharding: data-parallel over batch B=32 across 8 NeuronCores (4 batches/core).

v2 design (vs v1):
  - Permuted t-layout t = 8p + i (partition-major): the whole out row block of
    a partition is contiguous in HBM, so each batch's output is ONE DMA with
    128 x 32KB descriptors instead of ~3k 1-2KB lines.  All per-t math is
    permutation-invariant; only the T-sums (AC, s_tot) mix t and they are
    order-free.
  - Everything is staged in one [128, NT, 4D] tile per batch
    (cols 0:256 = C landed by the input DMA, 256:512 aq, 512:768 o3=C*aq,
    768:1024 o4=C*AC).
  - E' = exp(g + s_q + s_c) holds the FULL similarity: s_q enters as the ACT
    bias column of the exp, s_c enters via a rank-1 PE matmul
    (ones_row^T x sc_row) accumulated into the g PSUM bank.  Then
    n[t] = colmax_j E' directly (no exp(s_c) row pass, no n-row muls), and
    the attn normalization is unchanged (the exp(s_c[t]) factor cancels in
    U[:,0:256]/U[:,256]).
  - n columns come from GpSimd partition_all_reduce(max) + a diagonal
    extract (gpsimd mul by a replicated identity, DVE reduce_max) -- no
    more per-128 N=1 PE transposes of the n row.
  - U = E' @ [Q | 1] as ONE N=257 fp32r matmul per t-tile (ones column baked
    into Qaug); AC = n^T @ [C | 1] with the ones column baked into Cr, so
    s_tot falls out of the same accumulation (no separate sum matmul).
  - AC is accumulated eagerly per 512-chunk so only a short tail remains
    after the last chunk; the tail of batch b is issued inside batch b+1.
  - Elementwise work is spread: ACT (exp, aq-even, copies), DVE (aq-odd, o4,
    reciprocals, diag reduce), GpSimd (all_reduce, diag mul, o3, broadcast).
  - C of batch b+1 is prefetched during batch b (stage pool bufs=3).
"""

import numpy as np

import concourse.bass as bass
import concourse.mybir as mybir
import concourse.tile as tile
from concourse import bacc
from concourse import bass_isa
from concourse.bass_utils import run_bass_kernel_spmd
from concourse.masks import make_identity

F32 = mybir.dt.float32
F32R = mybir.dt.float32r
AF = mybir.ActivationFunctionType
ALU = mybir.AluOpType
AX = mybir.AxisListType

B, T, J, D = 32, 1024, 128, 256
NCORES = 8
BPC = B // NCORES      # batches per core
NT = T // 128          # t-tiles per batch
ND = D // 128          # d-tiles
NCH = T // 512         # 512-wide chunks per batch
TPC = 4                # t-tiles per chunk

USE_F32R = True


def _rr(ap):
    """float32r view of an f32 AP (for rounded producers + matmul operands)."""
    return ap.bitcast(F32R) if USE_F32R else ap


def _bcast_row(ap_1d, nparts):
    """DRAM AP [n] -> [nparts, n] with partition stride 0 (DMA broadcast)."""
    return bass.AP(
        tensor=ap_1d.tensor, offset=ap_1d.offset, ap=[[0, nparts]] + list(ap_1d.ap)
    )


def _fbcast(ap_col, n):
    """[128,1] column AP -> [128, n] with free stride 0."""
    return bass.AP(
        tensor=ap_col.tensor, offset=ap_col.offset,
        ap=[list(ap_col.ap)[0], [0, n]],
    )


def _tile_bcast(ap_2d, reps):
    """[128, n] AP -> [128, reps, n] with 0-stride middle dim."""
    a = list(ap_2d.ap)
    return bass.AP(
        tensor=ap_2d.tensor, offset=ap_2d.offset,
        ap=[a[0], [0, reps]] + a[1:],
    )


def build_nc(use_f32r=None):
    global USE_F32R
    if use_f32r is not None:
        USE_F32R = use_f32r
    nc = bacc.Bacc()
    ctx_in = nc.declare_dram_parameter("context", [BPC, T, D], F32, isOutput=False)
    qry_in = nc.declare_dram_parameter("query", [BPC, J, D], F32, isOutput=False)
    w_in = nc.declare_dram_parameter("w", [3 * D], F32, isOutput=False)
    out_ext = nc.declare_dram_parameter("out", [BPC, T, 4 * D], F32, isOutput=True)

    with tile.TileContext(nc) as tc:
        _body(tc, ctx_in, qry_in, w_in, out_ext)
    nc.finalize()
    return nc


def _body(tc, ctx_in, qry_in, w_in, out_ext):
    nc = tc.nc
    from contextlib import ExitStack

    with ExitStack() as ctx:
        consts = ctx.enter_context(tc.tile_pool(name="consts", bufs=1))
        stage_p = ctx.enter_context(tc.tile_pool(name="stage", bufs=4))
        big = ctx.enter_context(tc.tile_pool(name="big", bufs=2))
        work = ctx.enter_context(tc.tile_pool(name="work", bufs=2))
        tmp = ctx.enter_context(tc.tile_pool(name="tmp", bufs=1))
        # PSUM budget (8 banks): tr 2 + g 2 + sc 1 + u 2 + ac 1 = 8
        ps_tr = ctx.enter_context(tc.tile_pool(name="ps_tr", bufs=2, space="PSUM"))
        ps_g = ctx.enter_context(tc.tile_pool(name="ps_g", bufs=2, space="PSUM"))
        ps_sc = ctx.enter_context(tc.tile_pool(name="ps_sc", bufs=1, space="PSUM"))
        ps_u = ctx.enter_context(tc.tile_pool(name="ps_u", bufs=2, space="PSUM"))
        ps_ac = ctx.enter_context(tc.tile_pool(name="ps_ac", bufs=1, space="PSUM"))

        loads = {}

        def load_batch(bb, nsplit):
            st = stage_p.tile([128, NT, 4 * D], F32, tag="stage")
            qt = work.tile([128, D], F32, tag="Q")
            # Q first: it gates the first PE work (Q^T transposes)
            nc.sync.dma_start(out=qt, in_=qry_in[bb])
            src = ctx_in[bb].rearrange("(p i) d -> p i d", i=NT)
            step = NT // nsplit
            for s in range(nsplit):
                nc.sync.dma_start(
                    out=st[:, s * step : (s + 1) * step, 0:D],
                    in_=src[:, s * step : (s + 1) * step, :],
                )
            loads[bb] = (st, qt)

        # batch-0 input DMAs dispatched before all the consts traffic (v7)
        load_batch(0, 4)

        # --- constants (identity first: it gates the first PE transposes) ---
        ident = consts.tile([128, 128], F32)
        make_identity(nc, ident)
        ident4 = consts.tile([128, TPC, 128], F32)
        for j in range(TPC):
            nc.scalar.copy(ident4[:, j, :], ident)
        ones_col = consts.tile([128, 1], F32)
        nc.vector.memset(ones_col, 1.0)
        ones_row = consts.tile([1, 128], F32)
        nc.vector.memset(ones_row, 1.0)
        ones_row_r = consts.tile([1, 128], F32)
        nc.scalar.copy(_rr(ones_row_r), ones_row)

        # w_c / w_m as per-partition columns (two d-tiles each)
        wc_raw = consts.tile([128, ND], F32)
        wm_cols = consts.tile([128, ND], F32)
        for k in range(ND):
            nc.gpsimd.dma_start(
                out=wc_raw[:, k : k + 1],
                in_=w_in[k * 128 : (k + 1) * 128].rearrange("(p o) -> p o", o=1),
            )
            nc.gpsimd.dma_start(
                out=wm_cols[:, k : k + 1],
                in_=w_in[2 * D + k * 128 : 2 * D + (k + 1) * 128].rearrange(
                    "(p o) -> p o", o=1
                ),
            )
        # rounded copy so the s_c row matmuls can run in fp32r
        wc_cols = consts.tile([128, ND], F32)
        nc.scalar.copy(_rr(wc_cols), wc_raw)
        # w_q broadcast to all partitions (for s_q = rowsum(Q * w_q))
        wq_b = consts.tile([128, D], F32)
        nc.gpsimd.dma_start(out=wq_b, in_=_bcast_row(w_in[D : 2 * D], 128))

        def beta_tail_head(S):
            """Deferred AC matmuls + s_tot -> ac_row -> broadcast for batch
            S, issued after the NEXT batch's transposes so the n_all chain
            has a full block of PE work as cover (v7)."""
            b, st, n_all, Cr = S
            ac_ps = ps_ac.tile([1, D + 2], F32, tag="ac")
            for ii in range(NT):
                nc.tensor.matmul(
                    ac_ps,
                    lhsT=_rr(n_all[:, ii : ii + 1]),
                    rhs=_rr(Cr[:, ii, :]),
                    start=(ii == 0),
                    stop=(ii == NT - 1),
                )
            r_s = work.tile([1, 1], F32, tag="r_s")
            nc.vector.reciprocal(r_s, ac_ps[0:1, D : D + 1])
            ac_row = work.tile([1, D], F32, tag="ac_row")
            nc.scalar.activation(ac_row, ac_ps[0:1, 0:D], AF.Copy, scale=r_s)
            acb = work.tile([128, D], F32, tag="acb")
            nc.gpsimd.partition_broadcast(acb, ac_row, channels=128)
            return acb

        def beta_tail_finish(S, acb):
            """o4 = C * AC (DVE) + staged output DMA, in i-halves so the
            first half's 2MB DMA fires while the second half computes (v8)."""
            b, st = S[0], S[1]
            out_r = out_ext[b].rearrange("(p i) d -> p i d", i=NT)
            h = NT // 2
            for s in range(2):
                sl = slice(s * h, (s + 1) * h)
                nc.vector.tensor_mul(
                    st[:, sl, 3 * D : 4 * D], st[:, sl, 0:D], _tile_bcast(acb, h)
                )
                nc.sync.dma_start(out=out_r[:, sl, :], in_=st[:, sl, :])

        prev = None
        for b in range(BPC):
            if b + 1 < BPC:
                load_batch(b + 1, 2)
            st, Q = loads.pop(b)

            # ---- Q^T, with w_m folded in: qwT[d, j] = Q[j, d] * w_m[d] ----
            qt_ps = ps_tr.tile([128, D], F32, tag="tr")
            for k in range(ND):
                nc.tensor.transpose(
                    qt_ps[:, k * 128 : (k + 1) * 128], Q[:, k * 128 : (k + 1) * 128],
                    ident,
                )
            qwT = work.tile([128, D], F32, tag="qwT")
            for k in range(ND):
                nc.scalar.activation(
                    _rr(qwT[:, k * 128 : (k + 1) * 128]),
                    qt_ps[:, k * 128 : (k + 1) * 128],
                    AF.Copy,
                    scale=wm_cols[:, k : k + 1],
                )

            # ---- Qaug = [Q | 1] rounded (rhs of the U matmuls) ----
            Qaug = work.tile([128, D + 2], F32, tag="Qaug")
            nc.scalar.copy(_rr(Qaug[:, 0:D]), Q)
            nc.vector.tensor_copy(
                _rr(Qaug[:, D : D + 2]), _fbcast(ones_col, 2)
            )

            # ---- s_q column: rowsum(Q * w_q) ----
            sq_scr = tmp.tile([128, D], F32, tag="sq_scr")
            sq_col = work.tile([128, 1], F32, tag="sq_col")
            nc.vector.tensor_mul(sq_scr, Q, wq_b)
            nc.vector.reduce_sum(out=sq_col, in_=sq_scr, axis=AX.X)

            # ---- C^T via PE transposes ----
            ctT = big.tile([128, ND, NT, 128], F32, tag="ctT")
            for i2 in range(NT // 2):
                ct_ps = ps_tr.tile([128, 2 * ND * 128], F32, tag="tr")
                for u in range(2):
                    i = 2 * i2 + u
                    for k in range(ND):
                        nc.tensor.transpose(
                            ct_ps[:, (2 * u + k) * 128 : (2 * u + k + 1) * 128],
                            st[:, i, k * 128 : (k + 1) * 128],
                            ident,
                        )
                dst = _rr(ctT[:, :, 2 * i2 : 2 * i2 + 2, :])
                srcv = ct_ps.rearrange("p (t k x) -> p k t x", t=2, k=ND)
                if i2 % 2 == 0:
                    nc.scalar.copy(dst, srcv)
                else:
                    nc.vector.tensor_copy(dst, srcv)

            # ---- previous batch: deferred AC + s_tot/ac_row/bcast (v7) ----
            acb_prev = beta_tail_head(prev) if prev is not None else None

            # ---- per 512-chunk: g + s_c row matmuls ----
            ET = big.tile([128, T], F32, tag="ET")
            n_all = work.tile([128, NT], F32, tag="n_all")
            g_list = []
            for c in range(NCH):
                g_ps = ps_g.tile([128, 512], F32, tag="g")
                scp = ps_sc.tile([1, 512], F32, tag="sc")
                for k in range(ND):
                    nc.tensor.matmul(
                        g_ps,
                        lhsT=_rr(qwT[:, k * 128 : (k + 1) * 128]),
                        rhs=_rr(ctT[:, k, TPC * c : TPC * (c + 1), :]),
                        start=(k == 0),
                        stop=False,
                        skip_group_check=True,
                    )
                for k in range(ND):
                    nc.tensor.matmul(
                        scp,
                        lhsT=_rr(wc_cols[:, k : k + 1]),
                        rhs=_rr(ctT[:, k, TPC * c : TPC * (c + 1), :]),
                        start=(k == 0),
                        stop=(k == ND - 1),
                    )
                sc_row = tmp.tile([1, 512], F32, tag=f"sc_row{c}")
                nc.vector.tensor_copy(_rr(sc_row), scp)
                g_list.append((g_ps, sc_row))

            # ---- Cr = [C | 1] rounded (rhs of the AC matmuls) ----
            Cr = big.tile([128, NT, D + 2], F32, tag="Cr")
            nc.vector.tensor_copy(
                _rr(Cr[:, :, D : D + 2]),
                bass.AP(tensor=ones_col.tensor, offset=ones_col.offset,
                        ap=[list(ones_col.ap)[0], [0, NT], [0, 2]]),
            )
            h = NT // 2
            nc.scalar.copy(_rr(Cr[:, 0:h, 0:D]), st[:, 0:h, 0:D])
            nc.vector.tensor_copy(_rr(Cr[:, h:NT, 0:D]), st[:, h:NT, 0:D])

            # ---- per chunk: fold s_c (rank-1), E' = exp, colmax, diag ----
            # o4 + out-DMA of the previous batch are issued between the two
            # chunks so the DVE has work while the GpSimd all_reduce runs.
            me_list = []
            for c in range(NCH):
                g_ps, sc_row = g_list[c]
                nc.tensor.matmul(
                    g_ps,
                    lhsT=_rr(ones_row_r),
                    rhs=_rr(sc_row),
                    start=False,
                    stop=True,
                    skip_group_check=True,
                )
                nc.scalar.activation(
                    _rr(ET[:, c * 512 : (c + 1) * 512]), g_ps, AF.Exp, bias=sq_col
                )
                me = tmp.tile([128, 512], F32, tag=f"me{c}")
                # the tail chunk's reduce goes in halves so the final beta
                # chain is ~1.3us shorter on the last batch (v8)
                nh = 2 if (b == BPC - 1 and c == NCH - 1) else 1
                for hh in range(nh):
                    w512 = 512 // nh
                    nc.gpsimd.partition_all_reduce(
                        me[:, hh * w512 : (hh + 1) * w512],
                        ET[:, c * 512 + hh * w512 : c * 512 + (hh + 1) * w512],
                        channels=128,
                        reduce_op=bass_isa.ReduceOp.max,
                    )
                me_list.append((me, nh))
            for c in range(NCH):
                me, nh = me_list[c]
                scr = tmp.tile([128, TPC, 128], F32, tag=f"scr{c}")
                for hh in range(nh):
                    tp = TPC // nh
                    nc.vector.tensor_mul(
                        scr[:, hh * tp : (hh + 1) * tp, :],
                        me.rearrange("p (i r) -> p i r", r=128)[
                            :, hh * tp : (hh + 1) * tp, :
                        ],
                        ident4[:, 0:tp, :],
                    )
                    nc.vector.reduce_max(
                        out=_rr(
                            n_all[:, TPC * c + hh * tp : TPC * c + (hh + 1) * tp]
                        ),
                        in_=scr[:, hh * tp : (hh + 1) * tp, :],
                        axis=AX.X,
                    )

            # ---- per t-tile: U = E' @ [Q|1]; aq on ACT; o3 on DVE ----
            r_all = work.tile([128, NT], F32, tag="r_all")
            for i in range(NT):
                u_ps = ps_u.tile([128, D + 2], F32, tag="u")
                nc.tensor.matmul(
                    u_ps, lhsT=_rr(ET[:, i * 128 : (i + 1) * 128]), rhs=_rr(Qaug)
                )
                nc.vector.reciprocal(r_all[:, i : i + 1], u_ps[:, D : D + 1])
                nc.scalar.activation(
                    st[:, i, D : 2 * D], u_ps[:, 0:D], AF.Copy,
                    scale=r_all[:, i : i + 1],
                )
                if i % TPC == TPC - 1:
                    # o3 for this chunk's 4 tiles (one DVE pass)
                    j0 = i - (TPC - 1)
                    nc.vector.tensor_mul(
                        st[:, j0 : i + 1, 2 * D : 3 * D],
                        st[:, j0 : i + 1, D : 2 * D],
                        st[:, j0 : i + 1, 0:D],
                    )

            # ---- previous batch: o4 + output DMA (after the U loop so the
            #      DVE recips/aq aren't stuck behind the big o4 pass) ----
            if prev is not None:
                beta_tail_finish(prev, acb_prev)
            prev = (b, st, n_all, Cr)

        beta_tail_finish(prev, beta_tail_head(prev))


_NC_CACHE = {}


def kernel(context, query, w, b, _trace=False):
    context = np.ascontiguousarray(context, dtype=np.float32)
    query = np.ascontiguousarray(query, dtype=np.float32)
    w = np.ascontiguousarray(w, dtype=np.float32)

    if "nc" not in _NC_CACHE:
        _NC_CACHE["nc"] = build_nc()
    nc = _NC_CACHE["nc"]

    in_maps = [
        {
            "context": context[i * BPC : (i + 1) * BPC],
            "query": query[i * BPC : (i + 1) * BPC],
            "w": w,
        }
        for i in range(NCORES)
    ]
    try:
        res = run_bass_kernel_spmd(
            nc, in_maps, core_ids=list(range(NCORES)), trace=_trace
        )
    except Exception:
        # A previous process may have left the device wedged; reset and retry.
        import ctypes

        import jax

        jax.devices()
        lib = ctypes.CDLL("/opt/axon/libaxon_pjrt.so")
        if hasattr(lib, "axon_reset"):
            lib.axon_reset()
        res = run_bass_kernel_spmd(
            nc, in_maps, core_ids=list(range(NCORES)), trace=_trace
        )
    out = np.concatenate([res.results[i]["out"] for i in range(NCORES)], axis=0)
    if _trace:
        kernel.last_exec_time_ns = res.exec_time_ns
        kernel.last_results = res
    return out


if __name__ == "__main__":
    rng = np.random.default_rng(0)
    inputs = {
        "context": rng.standard_normal((B, T, D), dtype=np.float32),
        "query": rng.standard_normal((B, J, D), dtype=np.float32),
        "w": (rng.standard_normal(3 * D).astype(np.float32) / np.sqrt(3 * D)),
        "b": np.zeros(1, np.float32),
    }
    out = kernel(**inputs)
    print("out", out.shape, out.dtype, float(np.abs(out).mean()))


# revision 16
# speedup vs baseline: 1.1232x; 1.0240x over previous
"""AttentionFlow Trainium2 Bass kernel (v2).

Math (per batch):
  d = 256; w = [w_c | w_q | w_m]
  sim[t,j] = s_c[t] + s_q[j] + sum_d C[t,d] w_m[d] Q[j,d]   (+b, which cancels)
  attn = softmax_j(sim);  AQ = attn @ Q
  beta = softmax_t(max_j sim);  AC = beta @ C
  out = concat([C, AQ, C*AQ, C*AC], axis=-1)

Sharding: data-parallel over batch B=32 across 8 NeuronCores (4 batches/core).

v2 design (vs v1):
  - Permuted t-layout t = 8p + i (partition-major): the whole out row block of
    a partition is contiguous in HBM, so each batch's output is ONE DMA with
    128 x 32KB descriptors instead of ~3k 1-2KB lines.  All per-t math is
    permutation-invariant; only the T-sums (AC, s_tot) mix t and they are
    order-free.
  - Everything is staged in one [128, NT, 4D] tile per batch
    (cols 0:256 = C landed by the input DMA, 256:512 aq, 512:768 o3=C*aq,
    768:1024 o4=C*AC).
  - E' = exp(g + s_q + s_c) holds the FULL similarity: s_q enters as the ACT
    bias column of the exp, s_c enters via a rank-1 PE matmul
    (ones_row^T x sc_row) accumulated into the g PSUM bank.  Then
    n[t] = colmax_j E' directly (no exp(s_c) row pass, no n-row muls), and
    the attn normalization is unchanged (the exp(s_c[t]) factor cancels in
    U[:,0:256]/U[:,256]).
  - n columns come from GpSimd partition_all_reduce(max) + a diagonal
    extract (gpsimd mul by a replicated identity, DVE reduce_max) -- no
    more per-128 N=1 PE transposes of the n row.
  - U = E' @ [Q | 1] as ONE N=257 fp32r matmul per t-tile (ones column baked
    into Qaug); AC = n^T @ [C | 1] with the ones column baked into Cr, so
    s_tot falls out of the same accumulation (no separate sum matmul).
  - AC is accumulated eagerly per 512-chunk so only a short tail remains
    after the last chunk; the tail of batch b is issued inside batch b+1.
  - Elementwise work is spread: ACT (exp, aq-even, copies), DVE (aq-odd, o4,
    reciprocals, diag reduce), GpSimd (all_reduce, diag mul, o3, broadcast).
  - C of batch b+1 is prefetched during batch b (stage pool bufs=3).
"""

import numpy as np

import concourse.bass as bass
import concourse.mybir as mybir
import concourse.tile as tile
from concourse import bacc
from concourse import bass_isa
from concourse.bass_utils import run_bass_kernel_spmd
from concourse.masks import make_identity

F32 = mybir.dt.float32
F32R = mybir.dt.float32r
AF = mybir.ActivationFunctionType
ALU = mybir.AluOpType
AX = mybir.AxisListType

B, T, J, D = 32, 1024, 128, 256
NCORES = 8
BPC = B // NCORES      # batches per core
NT = T // 128          # t-tiles per batch
ND = D // 128          # d-tiles
NCH = T // 512         # 512-wide chunks per batch
TPC = 4                # t-tiles per chunk

USE_F32R = True


def _rr(ap):
    """float32r view of an f32 AP (for rounded producers + matmul operands)."""
    return ap.bitcast(F32R) if USE_F32R else ap


def _bcast_row(ap_1d, nparts):
    """DRAM AP [n] -> [nparts, n] with partition stride 0 (DMA broadcast)."""
    return bass.AP(
        tensor=ap_1d.tensor, offset=ap_1d.offset, ap=[[0, nparts]] + list(ap_1d.ap)
    )


def _fbcast(ap_col, n):
    """[128,1] column AP -> [128, n] with free stride 0."""
    return bass.AP(
        tensor=ap_col.tensor, offset=ap_col.offset,
        ap=[list(ap_col.ap)[0], [0, n]],
    )


def _tile_bcast(ap_2d, reps):
    """[128, n] AP -> [128, reps, n] with 0-stride middle dim."""
    a = list(ap_2d.ap)
    return bass.AP(
        tensor=ap_2d.tensor, offset=ap_2d.offset,
        ap=[a[0], [0, reps]] + a[1:],
    )


def build_nc(use_f32r=None):
    global USE_F32R
    if use_f32r is not None:
        USE_F32R = use_f32r
    nc = bacc.Bacc()
    ctx_in = nc.declare_dram_parameter("context", [BPC, T, D], F32, isOutput=False)
    qry_in = nc.declare_dram_parameter("query", [BPC, J, D], F32, isOutput=False)
    w_in = nc.declare_dram_parameter("w", [3 * D], F32, isOutput=False)
    out_ext = nc.declare_dram_parameter("out", [BPC, T, 4 * D], F32, isOutput=True)

    with tile.TileContext(nc) as tc:
        _body(tc, ctx_in, qry_in, w_in, out_ext)
    nc.finalize()
    return nc


def _body(tc, ctx_in, qry_in, w_in, out_ext):
    nc = tc.nc
    from contextlib import ExitStack

    with ExitStack() as ctx:
        consts = ctx.enter_context(tc.tile_pool(name="consts", bufs=1))
        stage_p = ctx.enter_context(tc.tile_pool(name="stage", bufs=4))
        big = ctx.enter_context(tc.tile_pool(name="big", bufs=2))
        work = ctx.enter_context(tc.tile_pool(name="work", bufs=2))
        tmp = ctx.enter_context(tc.tile_pool(name="tmp", bufs=1))
        # PSUM budget (8 banks): tr 2 + g 2 + sc 1 + u 2 + ac 1 = 8
        ps_tr = ctx.enter_context(tc.tile_pool(name="ps_tr", bufs=2, space="PSUM"))
        ps_g = ctx.enter_context(tc.tile_pool(name="ps_g", bufs=2, space="PSUM"))
        ps_sc = ctx.enter_context(tc.tile_pool(name="ps_sc", bufs=1, space="PSUM"))
        ps_u = ctx.enter_context(tc.tile_pool(name="ps_u", bufs=2, space="PSUM"))
        ps_ac = ctx.enter_context(tc.tile_pool(name="ps_ac", bufs=1, space="PSUM"))

        loads = {}

        def load_batch(bb, nsplit):
            st = stage_p.tile([128, NT, 4 * D], F32, tag="stage")
            qt = work.tile([128, D], F32, tag="Q")
            # Q first: it gates the first PE work (Q^T transposes)
            nc.sync.dma_start(out=qt, in_=qry_in[bb])
            src = ctx_in[bb].rearrange("(p i) d -> p i d", i=NT)
            step = NT // nsplit
            for s in range(nsplit):
                nc.sync.dma_start(
                    out=st[:, s * step : (s + 1) * step, 0:D],
                    in_=src[:, s * step : (s + 1) * step, :],
                )
            loads[bb] = (st, qt)

        # batch-0 input DMAs dispatched before all the consts traffic (v7)
        load_batch(0, 4)

        # --- constants (identity first: it gates the first PE transposes) ---
        ident = consts.tile([128, 128], F32)
        make_identity(nc, ident)
        ident4 = consts.tile([128, TPC, 128], F32)
        for j in range(TPC):
            nc.scalar.copy(ident4[:, j, :], ident)
        ones_col = consts.tile([128, 1], F32)
        nc.vector.memset(ones_col, 1.0)
        ones_row = consts.tile([1, 128], F32)
        nc.vector.memset(ones_row, 1.0)
        ones_row_r = consts.tile([1, 128], F32)
        nc.scalar.copy(_rr(ones_row_r), ones_row)

        # w_c / w_m as per-partition columns (two d-tiles each)
        wc_raw = consts.tile([128, ND], F32)
        wm_cols = consts.tile([128, ND], F32)
        for k in range(ND):
            nc.gpsimd.dma_start(
                out=wc_raw[:, k : k + 1],
                in_=w_in[k * 128 : (k + 1) * 128].rearrange("(p o) -> p o", o=1),
            )
            nc.gpsimd.dma_start(
                out=wm_cols[:, k : k + 1],
                in_=w_in[2 * D + k * 128 : 2 * D + (k + 1) * 128].rearrange(
                    "(p o) -> p o", o=1
                ),
            )
        # rounded copy so the s_c row matmuls can run in fp32r
        wc_cols = consts.tile([128, ND], F32)
        nc.scalar.copy(_rr(wc_cols), wc_raw)
        # w_q broadcast to all partitions (for s_q = rowsum(Q * w_q))
        wq_b = consts.tile([128, D], F32)
        nc.gpsimd.dma_start(out=wq_b, in_=_bcast_row(w_in[D : 2 * D], 128))

        def beta_tail_head(S):
            """Deferred AC matmuls + s_tot -> ac_row -> broadcast for batch
            S, issued after the NEXT batch's transposes so the n_all chain
            has a full block of PE work as cover (v7)."""
            b, st, n_all, Cr = S
            ac_ps = ps_ac.tile([1, D + 2], F32, tag="ac")
            for ii in range(NT):
                nc.tensor.matmul(
                    ac_ps,
                    lhsT=_rr(n_all[:, ii : ii + 1]),
                    rhs=_rr(Cr[:, ii, :]),
                    start=(ii == 0),
                    stop=(ii == NT - 1),
                )
            r_s = work.tile([1, 1], F32, tag="r_s")
            nc.vector.reciprocal(r_s, ac_ps[0:1, D : D + 1])
            ac_row = work.tile([1, D], F32, tag="ac_row")
            nc.scalar.activation(ac_row, ac_ps[0:1, 0:D], AF.Copy, scale=r_s)
            acb = work.tile([128, D], F32, tag="acb")
            nc.gpsimd.partition_broadcast(acb, ac_row, channels=128)
            return acb

        def beta_tail_finish(S, acb):
            """o4 = C * AC (DVE) + staged output DMA, in i-halves so the
            first half's 2MB DMA fires while the second half computes (v8)."""
            b, st = S[0], S[1]
            out_r = out_ext[b].rearrange("(p i) d -> p i d", i=NT)
            h = NT // 2
            for s in range(2):
                sl = slice(s * h, (s + 1) * h)
                nc.vector.tensor_mul(
                    st[:, sl, 3 * D : 4 * D], st[:, sl, 0:D], _tile_bcast(acb, h)
                )
                nc.sync.dma_start(out=out_r[:, sl, :], in_=st[:, sl, :])

        prev = None
        for b in range(BPC):
            if b + 1 < BPC:
                load_batch(b + 1, 2)
            st, Q = loads.pop(b)

            # ---- Q^T, with w_m folded in: qwT[d, j] = Q[j, d] * w_m[d] ----
            qt_ps = ps_tr.tile([128, D], F32, tag="tr")
            for k in range(ND):
                nc.tensor.transpose(
                    qt_ps[:, k * 128 : (k + 1) * 128], Q[:, k * 128 : (k + 1) * 128],
                    ident,
                )
            qwT = work.tile([128, D], F32, tag="qwT")
            for k in range(ND):
                nc.scalar.activation(
                    _rr(qwT[:, k * 128 : (k + 1) * 128]),
                    qt_ps[:, k * 128 : (k + 1) * 128],
                    AF.Copy,
                    scale=wm_cols[:, k : k + 1],
                )

            # ---- Qaug = [Q | 1] rounded (rhs of the U matmuls) ----
            Qaug = work.tile([128, D + 2], F32, tag="Qaug")
            nc.scalar.copy(_rr(Qaug[:, 0:D]), Q)
            nc.vector.tensor_copy(
                _rr(Qaug[:, D : D + 2]), _fbcast(ones_col, 2)
            )

            # ---- s_q column: rowsum(Q * w_q) ----
            sq_scr = tmp.tile([128, D], F32, tag="sq_scr")
            sq_col = work.tile([128, 1], F32, tag="sq_col")
            nc.vector.tensor_mul(sq_scr, Q, wq_b)
            nc.vector.reduce_sum(out=sq_col, in_=sq_scr, axis=AX.X)

            # ---- C^T via PE transposes ----
            ctT = big.tile([128, ND, NT, 128], F32, tag="ctT")
            for i2 in range(NT // 2):
                ct_ps = ps_tr.tile([128, 2 * ND * 128], F32, tag="tr")
                for u in range(2):
                    i = 2 * i2 + u
                    for k in range(ND):
                        nc.tensor.transpose(
                            ct_ps[:, (2 * u + k) * 128 : (2 * u + k + 1) * 128],
                            st[:, i, k * 128 : (k + 1) * 128],
                            ident,
                        )
                dst = _rr(ctT[:, :, 2 * i2 : 2 * i2 + 2, :])
                srcv = ct_ps.rearrange("p (t k x) -> p k t x", t=2, k=ND)
                if i2 % 2 == 0:
                    nc.scalar.copy(dst, srcv)
                else:
                    nc.vector.tensor_copy(dst, srcv)

            # ---- previous batch: deferred AC + s_tot/ac_row/bcast (v7) ----
            acb_prev = beta_tail_head(prev) if prev is not None else None

            # ---- per 512-chunk: g + s_c row matmuls ----
            ET = big.tile([128, T], F32, tag="ET")
            n_all = work.tile([128, NT], F32, tag="n_all")
            g_list = []
            for c in range(NCH):
                g_ps = ps_g.tile([128, 512], F32, tag="g")
                scp = ps_sc.tile([1, 512], F32, tag="sc")
                for k in range(ND):
                    nc.tensor.matmul(
                        g_ps,
                        lhsT=_rr(qwT[:, k * 128 : (k + 1) * 128]),
                        rhs=_rr(ctT[:, k, TPC * c : TPC * (c + 1), :]),
                        start=(k == 0),
                        stop=False,
                        skip_group_check=True,
                    )
                for k in range(ND):
                    nc.tensor.matmul(
                        scp,
                        lhsT=_rr(wc_cols[:, k : k + 1]),
                        rhs=_rr(ctT[:, k, TPC * c : TPC * (c + 1), :]),
                        start=(k == 0),
                        stop=(k == ND - 1),
                    )
                sc_row = tmp.tile([1, 512], F32, tag=f"sc_row{c}")
                nc.vector.tensor_copy(_rr(sc_row), scp)
                g_list.append((g_ps, sc_row))

            # ---- Cr = [C | 1] rounded (rhs of the AC matmuls) ----
            Cr = big.tile([128, NT, D + 2], F32, tag="Cr")
            nc.vector.tensor_copy(
                _rr(Cr[:, :, D : D + 2]),
                bass.AP(tensor=ones_col.tensor, offset=ones_col.offset,
                        ap=[list(ones_col.ap)[0], [0, NT], [0, 2]]),
            )
            h = NT // 2
            nc.scalar.copy(_rr(Cr[:, 0:h, 0:D]), st[:, 0:h, 0:D])
            nc.vector.tensor_copy(_rr(Cr[:, h:NT, 0:D]), st[:, h:NT, 0:D])

            # ---- per chunk: fold s_c (rank-1), E' = exp, colmax, diag ----
            # o4 + out-DMA of the previous batch are issued between the two
            # chunks so the DVE has work while the GpSimd all_reduce runs.
            me_list = []
            for c in range(NCH):
                g_ps, sc_row = g_list[c]
                nc.tensor.matmul(
                    g_ps,
                    lhsT=_rr(ones_row_r),
                    rhs=_rr(sc_row),
                    start=False,
                    stop=True,
                    skip_group_check=True,
                )
                nc.scalar.activation(
                    _rr(ET[:, c * 512 : (c + 1) * 512]), g_ps, AF.Exp, bias=sq_col
                )
                me = tmp.tile([128, 512], F32, tag=f"me{c}")
                # the tail chunk's reduce goes in halves so the final beta
                # chain is ~1.3us shorter on the last batch (v8)
                nh = 2 if (b == BPC - 1 and c == NCH - 1) else 1
                for hh in range(nh):
                    w512 = 512 // nh
                    nc.gpsimd.partition_all_reduce(
                        me[:, hh * w512 : (hh + 1) * w512],
                        ET[:, c * 512 + hh * w512 : c * 512 + (hh + 1) * w512],
                        channels=128,
                        reduce_op=bass_isa.ReduceOp.max,
                    )
                me_list.append((me, nh))
            for c in range(NCH):
                me, nh = me_list[c]
                scr = tmp.tile([128, TPC, 128], F32, tag=f"scr{c}")
                mev = me.rearrange("p (i r) -> p i r", r=128)
                for hh in range(nh):
                    tp = TPC // nh
                    sl = slice(hh * tp, (hh + 1) * tp)
                    nc.vector.tensor_mul(scr[:, sl, :], mev[:, sl, :], ident4[:, 0:tp, :])
                    nc.vector.reduce_max(
                        out=_rr(n_all[:, TPC * c + hh * tp : TPC * c + (hh + 1) * tp]),
                        in_=scr[:, sl, :],
                        axis=AX.X,
                    )

            # ---- per t-tile: U = E' @ [Q|1]; aq on ACT; o3 on DVE ----
            r_all = work.tile([128, NT], F32, tag="r_all")
            for i in range(NT):
                u_ps = ps_u.tile([128, D + 2], F32, tag="u")
                nc.tensor.matmul(
                    u_ps, lhsT=_rr(ET[:, i * 128 : (i + 1) * 128]), rhs=_rr(Qaug)
                )
                nc.vector.reciprocal(r_all[:, i : i + 1], u_ps[:, D : D + 1])
                nc.scalar.activation(
                    st[:, i, D : 2 * D], u_ps[:, 0:D], AF.Copy,
                    scale=r_all[:, i : i + 1],
                )
                if i % TPC == TPC - 1:
                    # o3 for this chunk's 4 tiles (one DVE pass)
                    j0 = i - (TPC - 1)
                    nc.vector.tensor_mul(
                        st[:, j0 : i + 1, 2 * D : 3 * D],
                        st[:, j0 : i + 1, D : 2 * D],
                        st[:, j0 : i + 1, 0:D],
                    )

            # ---- previous batch: o4 + output DMA (after the U loop so the
            #      DVE recips/aq aren't stuck behind the big o4 pass) ----
            if prev is not None:
                beta_tail_finish(prev, acb_prev)
            prev = (b, st, n_all, Cr)

        beta_tail_finish(prev, beta_tail_head(prev))


_NC_CACHE = {}


def kernel(context, query, w, b, _trace=False):
    context = np.ascontiguousarray(context, dtype=np.float32)
    query = np.ascontiguousarray(query, dtype=np.float32)
    w = np.ascontiguousarray(w, dtype=np.float32)

    if "nc" not in _NC_CACHE:
        _NC_CACHE["nc"] = build_nc()
    nc = _NC_CACHE["nc"]

    in_maps = [
        {
            "context": context[i * BPC : (i + 1) * BPC],
            "query": query[i * BPC : (i + 1) * BPC],
            "w": w,
        }
        for i in range(NCORES)
    ]
    try:
        res = run_bass_kernel_spmd(
            nc, in_maps, core_ids=list(range(NCORES)), trace=_trace
        )
    except Exception:
        # A previous process may have left the device wedged; reset and retry.
        import ctypes

        import jax

        jax.devices()
        lib = ctypes.CDLL("/opt/axon/libaxon_pjrt.so")
        if hasattr(lib, "axon_reset"):
            lib.axon_reset()
        res = run_bass_kernel_spmd(
            nc, in_maps, core_ids=list(range(NCORES)), trace=_trace
        )
    out = np.concatenate([res.results[i]["out"] for i in range(NCORES)], axis=0)
    if _trace:
        kernel.last_exec_time_ns = res.exec_time_ns
        kernel.last_results = res
    return out


if __name__ == "__main__":
    rng = np.random.default_rng(0)
    inputs = {
        "context": rng.standard_normal((B, T, D), dtype=np.float32),
        "query": rng.standard_normal((B, J, D), dtype=np.float32),
        "w": (rng.standard_normal(3 * D).astype(np.float32) / np.sqrt(3 * D)),
        "b": np.zeros(1, np.float32),
    }
    out = kernel(**inputs)
    print("out", out.shape, out.dtype, float(np.abs(out).mean()))


# revision 17
# speedup vs baseline: 1.1727x; 1.0441x over previous
"""AttentionFlow Trainium2 Bass kernel (v2).

Math (per batch):
  d = 256; w = [w_c | w_q | w_m]
  sim[t,j] = s_c[t] + s_q[j] + sum_d C[t,d] w_m[d] Q[j,d]   (+b, which cancels)
  attn = softmax_j(sim);  AQ = attn @ Q
  beta = softmax_t(max_j sim);  AC = beta @ C
  out = concat([C, AQ, C*AQ, C*AC], axis=-1)

Sharding: data-parallel over batch B=32 across 8 NeuronCores (4 batches/core).

v2 design (vs v1):
  - Permuted t-layout t = 8p + i (partition-major): the whole out row block of
    a partition is contiguous in HBM, so each batch's output is ONE DMA with
    128 x 32KB descriptors instead of ~3k 1-2KB lines.  All per-t math is
    permutation-invariant; only the T-sums (AC, s_tot) mix t and they are
    order-free.
  - Everything is staged in one [128, NT, 4D] tile per batch
    (cols 0:256 = C landed by the input DMA, 256:512 aq, 512:768 o3=C*aq,
    768:1024 o4=C*AC).
  - E' = exp(g + s_q + s_c) holds the FULL similarity: s_q enters as the ACT
    bias column of the exp, s_c enters via a rank-1 PE matmul
    (ones_row^T x sc_row) accumulated into the g PSUM bank.  Then
    n[t] = colmax_j E' directly (no exp(s_c) row pass, no n-row muls), and
    the attn normalization is unchanged (the exp(s_c[t]) factor cancels in
    U[:,0:256]/U[:,256]).
  - n columns come from GpSimd partition_all_reduce(max) + a diagonal
    extract (gpsimd mul by a replicated identity, DVE reduce_max) -- no
    more per-128 N=1 PE transposes of the n row.
  - U = E' @ [Q | 1] as ONE N=257 fp32r matmul per t-tile (ones column baked
    into Qaug); AC = n^T @ [C | 1] with the ones column baked into Cr, so
    s_tot falls out of the same accumulation (no separate sum matmul).
  - AC is accumulated eagerly per 512-chunk so only a short tail remains
    after the last chunk; the tail of batch b is issued inside batch b+1.
  - Elementwise work is spread: ACT (exp, aq-even, copies), DVE (aq-odd, o4,
    reciprocals, diag reduce), GpSimd (all_reduce, diag mul, o3, broadcast).
  - C of batch b+1 is prefetched during batch b (stage pool bufs=3).
"""

import numpy as np

import concourse.bass as bass
import concourse.mybir as mybir
import concourse.tile as tile
from concourse import bacc
from concourse import bass_isa
from concourse.bass_utils import run_bass_kernel_spmd
from concourse.masks import make_identity

F32 = mybir.dt.float32
F32R = mybir.dt.float32r
AF = mybir.ActivationFunctionType
ALU = mybir.AluOpType
AX = mybir.AxisListType

B, T, J, D = 32, 1024, 128, 256
NCORES = 8
BPC = B // NCORES      # batches per core
NT = T // 128          # t-tiles per batch
ND = D // 128          # d-tiles
NCH = T // 512         # 512-wide chunks per batch
TPC = 4                # t-tiles per chunk

USE_F32R = True


def _rr(ap):
    """float32r view of an f32 AP (for rounded producers + matmul operands)."""
    return ap.bitcast(F32R) if USE_F32R else ap


def _bcast_row(ap_1d, nparts):
    """DRAM AP [n] -> [nparts, n] with partition stride 0 (DMA broadcast)."""
    return bass.AP(
        tensor=ap_1d.tensor, offset=ap_1d.offset, ap=[[0, nparts]] + list(ap_1d.ap)
    )


def _fbcast(ap_col, n):
    """[128,1] column AP -> [128, n] with free stride 0."""
    return bass.AP(
        tensor=ap_col.tensor, offset=ap_col.offset,
        ap=[list(ap_col.ap)[0], [0, n]],
    )


def _tile_bcast(ap_2d, reps):
    """[128, n] AP -> [128, reps, n] with 0-stride middle dim."""
    a = list(ap_2d.ap)
    return bass.AP(
        tensor=ap_2d.tensor, offset=ap_2d.offset,
        ap=[a[0], [0, reps]] + a[1:],
    )


def build_nc(use_f32r=None):
    global USE_F32R
    if use_f32r is not None:
        USE_F32R = use_f32r
    nc = bacc.Bacc()
    ctx_in = nc.declare_dram_parameter("context", [BPC, T, D], F32, isOutput=False)
    qry_in = nc.declare_dram_parameter("query", [BPC, J, D], F32, isOutput=False)
    w_in = nc.declare_dram_parameter("w", [3 * D], F32, isOutput=False)
    out_ext = nc.declare_dram_parameter("out", [BPC, T, 4 * D], F32, isOutput=True)

    with tile.TileContext(nc) as tc:
        _body(tc, ctx_in, qry_in, w_in, out_ext)
    nc.finalize()
    return nc


def _body(tc, ctx_in, qry_in, w_in, out_ext):
    nc = tc.nc
    from contextlib import ExitStack

    with ExitStack() as ctx:
        consts = ctx.enter_context(tc.tile_pool(name="consts", bufs=1))
        stage_p = ctx.enter_context(tc.tile_pool(name="stage", bufs=4))
        big = ctx.enter_context(tc.tile_pool(name="big", bufs=2))
        work = ctx.enter_context(tc.tile_pool(name="work", bufs=2))
        tmp = ctx.enter_context(tc.tile_pool(name="tmp", bufs=1))
        # PSUM budget (8 banks): tr 2 + g 2 + u 3 + ac 1 = 8
        ps_tr = ctx.enter_context(tc.tile_pool(name="ps_tr", bufs=2, space="PSUM"))
        ps_g = ctx.enter_context(tc.tile_pool(name="ps_g", bufs=2, space="PSUM"))
        ps_u = ctx.enter_context(tc.tile_pool(name="ps_u", bufs=3, space="PSUM"))
        ps_ac = ctx.enter_context(tc.tile_pool(name="ps_ac", bufs=1, space="PSUM"))

        loads = {}

        def load_batch(bb, nsplit):
            st = stage_p.tile([128, NT, 4 * D], F32, tag="stage")
            qt = work.tile([128, D], F32, tag="Q")
            # Q first: it gates the first PE work (Q^T transposes)
            nc.sync.dma_start(out=qt, in_=qry_in[bb])
            src = ctx_in[bb].rearrange("(p i) d -> p i d", i=NT)
            step = NT // nsplit
            for s in range(nsplit):
                nc.sync.dma_start(
                    out=st[:, s * step : (s + 1) * step, 0:D],
                    in_=src[:, s * step : (s + 1) * step, :],
                )
            loads[bb] = (st, qt)

        # batch-0 input DMAs dispatched before all the consts traffic (v7)
        load_batch(0, 4)

        # --- constants (identity first: it gates the first PE transposes) ---
        ident = consts.tile([128, 128], F32)
        make_identity(nc, ident)
        ident4 = consts.tile([128, TPC, 128], F32)
        for j in range(TPC):
            nc.scalar.copy(ident4[:, j, :], ident)
        ones_col = consts.tile([128, 1], F32)
        nc.vector.memset(ones_col, 1.0)

        # w_c / w_m as per-partition columns (two d-tiles each)
        wc_raw = consts.tile([128, ND], F32)
        wm_cols = consts.tile([128, ND], F32)
        for k in range(ND):
            nc.gpsimd.dma_start(
                out=wc_raw[:, k : k + 1],
                in_=w_in[k * 128 : (k + 1) * 128].rearrange("(p o) -> p o", o=1),
            )
            nc.gpsimd.dma_start(
                out=wm_cols[:, k : k + 1],
                in_=w_in[2 * D + k * 128 : 2 * D + (k + 1) * 128].rearrange(
                    "(p o) -> p o", o=1
                ),
            )
        # wc replicated along free dim: (wc x ones) as a rank-1 stationary
        # adds s_c[t] to every j row inside the g accumulation (v11)
        wc_rep = consts.tile([128, ND, 128], F32)
        for k in range(ND):
            nc.scalar.copy(
                _rr(wc_rep[:, k, :]), _fbcast(wc_raw[:, k : k + 1], 128)
            )
        # w_q broadcast to all partitions (for s_q = rowsum(Q * w_q))
        wq_b = consts.tile([128, D], F32)
        nc.gpsimd.dma_start(out=wq_b, in_=_bcast_row(w_in[D : 2 * D], 128))

        def beta_tail_head(S):
            """Deferred AC matmuls + s_tot -> ac_row -> broadcast for batch
            S, issued after the NEXT batch's transposes so the n_all chain
            has a full block of PE work as cover (v7)."""
            b, st, n_all, Cr = S
            ac_ps = ps_ac.tile([1, D + 2], F32, tag="ac")
            for ii in range(NT):
                nc.tensor.matmul(
                    ac_ps,
                    lhsT=_rr(n_all[:, ii : ii + 1]),
                    rhs=_rr(Cr[:, ii, :]),
                    start=(ii == 0),
                    stop=(ii == NT - 1),
                )
            r_s = work.tile([1, 1], F32, tag="r_s")
            nc.vector.reciprocal(r_s, ac_ps[0:1, D : D + 1])
            ac_row = work.tile([1, D], F32, tag="ac_row")
            nc.scalar.activation(ac_row, ac_ps[0:1, 0:D], AF.Copy, scale=r_s)
            acb = work.tile([128, D], F32, tag="acb")
            nc.gpsimd.partition_broadcast(acb, ac_row, channels=128)
            return acb

        def beta_tail_finish(S, acb):
            """o4 = C * AC (DVE) + staged output DMA, in i-halves so the
            first half's 2MB DMA fires while the second half computes (v8)."""
            b, st = S[0], S[1]
            out_r = out_ext[b].rearrange("(p i) d -> p i d", i=NT)
            h = NT // 2
            for s in range(2):
                sl = slice(s * h, (s + 1) * h)
                nc.vector.tensor_mul(
                    st[:, sl, 3 * D : 4 * D], st[:, sl, 0:D], _tile_bcast(acb, h)
                )
                nc.sync.dma_start(out=out_r[:, sl, :], in_=st[:, sl, :])

        prev = None
        for b in range(BPC):
            if b + 1 < BPC:
                load_batch(b + 1, 2)
            st, Q = loads.pop(b)

            # ---- Q^T, with w_m folded in: qwT[d, j] = Q[j, d] * w_m[d] ----
            qt_ps = ps_tr.tile([128, D], F32, tag="tr")
            for k in range(ND):
                nc.tensor.transpose(
                    qt_ps[:, k * 128 : (k + 1) * 128], Q[:, k * 128 : (k + 1) * 128],
                    ident,
                )
            qwT = work.tile([128, D], F32, tag="qwT")
            for k in range(ND):
                nc.scalar.activation(
                    _rr(qwT[:, k * 128 : (k + 1) * 128]),
                    qt_ps[:, k * 128 : (k + 1) * 128],
                    AF.Copy,
                    scale=wm_cols[:, k : k + 1],
                )

            # ---- Qaug = [Q | 1] rounded (rhs of the U matmuls) ----
            Qaug = work.tile([128, D + 2], F32, tag="Qaug")
            nc.scalar.copy(_rr(Qaug[:, 0:D]), Q)
            nc.vector.tensor_copy(
                _rr(Qaug[:, D : D + 2]), _fbcast(ones_col, 2)
            )

            # ---- s_q column: rowsum(Q * w_q) ----
            sq_scr = tmp.tile([128, D], F32, tag="sq_scr")
            sq_col = work.tile([128, 1], F32, tag="sq_col")
            nc.vector.tensor_mul(sq_scr, Q, wq_b)
            nc.vector.reduce_sum(out=sq_col, in_=sq_scr, axis=AX.X)

            # ---- C^T via PE transposes ----
            ctT = big.tile([128, ND, NT, 128], F32, tag="ctT")
            for i2 in range(NT // 2):
                ct_ps = ps_tr.tile([128, 2 * ND * 128], F32, tag="tr")
                for u in range(2):
                    i = 2 * i2 + u
                    for k in range(ND):
                        nc.tensor.transpose(
                            ct_ps[:, (2 * u + k) * 128 : (2 * u + k + 1) * 128],
                            st[:, i, k * 128 : (k + 1) * 128],
                            ident,
                        )
                dst = _rr(ctT[:, :, 2 * i2 : 2 * i2 + 2, :])
                srcv = ct_ps.rearrange("p (t k x) -> p k t x", t=2, k=ND)
                if i2 % 2 == 0:
                    nc.scalar.copy(dst, srcv)
                else:
                    nc.vector.tensor_copy(dst, srcv)

            # ---- previous batch: deferred AC + s_tot/ac_row/bcast (v7) ----
            acb_prev = beta_tail_head(prev) if prev is not None else None

            # ---- per 512-chunk: g + s_c row matmuls ----
            ET = big.tile([128, T], F32, tag="ET")
            n_all = work.tile([128, NT], F32, tag="n_all")
            g_list = []
            for c in range(NCH):
                g_ps = ps_g.tile([128, 512], F32, tag="g")
                for k in range(ND):
                    nc.tensor.matmul(
                        g_ps,
                        lhsT=_rr(qwT[:, k * 128 : (k + 1) * 128]),
                        rhs=_rr(ctT[:, k, TPC * c : TPC * (c + 1), :]),
                        start=(k == 0),
                        stop=False,
                        skip_group_check=True,
                    )
                for k in range(ND):
                    nc.tensor.matmul(
                        g_ps,
                        lhsT=_rr(wc_rep[:, k, :]),
                        rhs=_rr(ctT[:, k, TPC * c : TPC * (c + 1), :]),
                        start=False,
                        stop=(k == ND - 1),
                        skip_group_check=True,
                    )
                g_list.append(g_ps)

            # ---- Cr = [C | 1] rounded (rhs of the AC matmuls) ----
            Cr = big.tile([128, NT, D + 2], F32, tag="Cr")
            nc.vector.tensor_copy(
                _rr(Cr[:, :, D : D + 2]),
                bass.AP(tensor=ones_col.tensor, offset=ones_col.offset,
                        ap=[list(ones_col.ap)[0], [0, NT], [0, 2]]),
            )
            h = NT // 2
            nc.scalar.copy(_rr(Cr[:, 0:h, 0:D]), st[:, 0:h, 0:D])
            nc.vector.tensor_copy(_rr(Cr[:, h:NT, 0:D]), st[:, h:NT, 0:D])

            # ---- per chunk: fold s_c (rank-1), E' = exp, colmax, diag ----
            # o4 + out-DMA of the previous batch are issued between the two
            # chunks so the DVE has work while the GpSimd all_reduce runs.
            me_list = []
            for c in range(NCH):
                g_ps = g_list[c]
                nc.scalar.activation(
                    _rr(ET[:, c * 512 : (c + 1) * 512]), g_ps, AF.Exp, bias=sq_col
                )
                me = tmp.tile([128, 512], F32, tag=f"me{c}")
                # the tail chunk's reduce goes in halves so the final beta
                # chain is ~1.3us shorter on the last batch (v8)
                nh = 2 if (b == BPC - 1 and c == NCH - 1) else 1
                for hh in range(nh):
                    w512 = 512 // nh
                    nc.gpsimd.partition_all_reduce(
                        me[:, hh * w512 : (hh + 1) * w512],
                        ET[:, c * 512 + hh * w512 : c * 512 + (hh + 1) * w512],
                        channels=128,
                        reduce_op=bass_isa.ReduceOp.max,
                    )
                me_list.append((me, nh))
            for c in range(NCH):
                me, nh = me_list[c]
                scr = tmp.tile([128, TPC, 128], F32, tag=f"scr{c}")
                mev = me.rearrange("p (i r) -> p i r", r=128)
                for hh in range(nh):
                    tp = TPC // nh
                    sl = slice(hh * tp, (hh + 1) * tp)
                    nc.vector.tensor_mul(scr[:, sl, :], mev[:, sl, :], ident4[:, 0:tp, :])
                    nc.vector.reduce_max(
                        out=_rr(n_all[:, TPC * c + hh * tp : TPC * c + (hh + 1) * tp]),
                        in_=scr[:, sl, :],
                        axis=AX.X,
                    )

            # ---- per t-tile: U = E' @ [Q|1]; aq on ACT; o3 on DVE ----
            r_all = work.tile([128, NT], F32, tag="r_all")
            for i in range(NT):
                u_ps = ps_u.tile([128, D + 2], F32, tag="u")
                nc.tensor.matmul(
                    u_ps, lhsT=_rr(ET[:, i * 128 : (i + 1) * 128]), rhs=_rr(Qaug)
                )
                nc.vector.reciprocal(r_all[:, i : i + 1], u_ps[:, D : D + 1])
                nc.scalar.activation(
                    st[:, i, D : 2 * D], u_ps[:, 0:D], AF.Copy,
                    scale=r_all[:, i : i + 1],
                )
                if i % TPC == TPC - 1:
                    # o3 for this chunk's 4 tiles (one DVE pass)
                    j0 = i - (TPC - 1)
                    nc.vector.tensor_mul(
                        st[:, j0 : i + 1, 2 * D : 3 * D],
                        st[:, j0 : i + 1, D : 2 * D],
                        st[:, j0 : i + 1, 0:D],
                    )

            # ---- previous batch: o4 + output DMA (after the U loop so the
            #      DVE recips/aq aren't stuck behind the big o4 pass) ----
            if prev is not None:
                beta_tail_finish(prev, acb_prev)
            prev = (b, st, n_all, Cr)

        beta_tail_finish(prev, beta_tail_head(prev))


_NC_CACHE = {}


def kernel(context, query, w, b, _trace=False):
    context = np.ascontiguousarray(context, dtype=np.float32)
    query = np.ascontiguousarray(query, dtype=np.float32)
    w = np.ascontiguousarray(w, dtype=np.float32)

    if "nc" not in _NC_CACHE:
        _NC_CACHE["nc"] = build_nc()
    nc = _NC_CACHE["nc"]

    in_maps = [
        {
            "context": context[i * BPC : (i + 1) * BPC],
            "query": query[i * BPC : (i + 1) * BPC],
            "w": w,
        }
        for i in range(NCORES)
    ]
    try:
        res = run_bass_kernel_spmd(
            nc, in_maps, core_ids=list(range(NCORES)), trace=_trace
        )
    except Exception:
        # A previous process may have left the device wedged; reset and retry.
        import ctypes

        import jax

        jax.devices()
        lib = ctypes.CDLL("/opt/axon/libaxon_pjrt.so")
        if hasattr(lib, "axon_reset"):
            lib.axon_reset()
        res = run_bass_kernel_spmd(
            nc, in_maps, core_ids=list(range(NCORES)), trace=_trace
        )
    out = np.concatenate([res.results[i]["out"] for i in range(NCORES)], axis=0)
    if _trace:
        kernel.last_exec_time_ns = res.exec_time_ns
        kernel.last_results = res
    return out


if __name__ == "__main__":
    rng = np.random.default_rng(0)
    inputs = {
        "context": rng.standard_normal((B, T, D), dtype=np.float32),
        "query": rng.standard_normal((B, J, D), dtype=np.float32),
        "w": (rng.standard_normal(3 * D).astype(np.float32) / np.sqrt(3 * D)),
        "b": np.zeros(1, np.float32),
    }
    out = kernel(**inputs)
    print("out", out.shape, out.dtype, float(np.abs(out).mean()))


# revision 18
# speedup vs baseline: 1.2031x; 1.0259x over previous
"""AttentionFlow Trainium2 Bass kernel (v2).

Math (per batch):
  d = 256; w = [w_c | w_q | w_m]
  sim[t,j] = s_c[t] + s_q[j] + sum_d C[t,d] w_m[d] Q[j,d]   (+b, which cancels)
  attn = softmax_j(sim);  AQ = attn @ Q
  beta = softmax_t(max_j sim);  AC = beta @ C
  out = concat([C, AQ, C*AQ, C*AC], axis=-1)

Sharding: data-parallel over batch B=32 across 8 NeuronCores (4 batches/core).

v2 design (vs v1):
  - Permuted t-layout t = 8p + i (partition-major): the whole out row block of
    a partition is contiguous in HBM, so each batch's output is ONE DMA with
    128 x 32KB descriptors instead of ~3k 1-2KB lines.  All per-t math is
    permutation-invariant; only the T-sums (AC, s_tot) mix t and they are
    order-free.
  - Everything is staged in one [128, NT, 4D] tile per batch
    (cols 0:256 = C landed by the input DMA, 256:512 aq, 512:768 o3=C*aq,
    768:1024 o4=C*AC).
  - E' = exp(g + s_q + s_c) holds the FULL similarity: s_q enters as the ACT
    bias column of the exp, s_c enters via a rank-1 PE matmul
    (ones_row^T x sc_row) accumulated into the g PSUM bank.  Then
    n[t] = colmax_j E' directly (no exp(s_c) row pass, no n-row muls), and
    the attn normalization is unchanged (the exp(s_c[t]) factor cancels in
    U[:,0:256]/U[:,256]).
  - n columns come from GpSimd partition_all_reduce(max) + a diagonal
    extract (gpsimd mul by a replicated identity, DVE reduce_max) -- no
    more per-128 N=1 PE transposes of the n row.
  - U = E' @ [Q | 1] as ONE N=257 fp32r matmul per t-tile (ones column baked
    into Qaug); AC = n^T @ [C | 1] with the ones column baked into Cr, so
    s_tot falls out of the same accumulation (no separate sum matmul).
  - AC is accumulated eagerly per 512-chunk so only a short tail remains
    after the last chunk; the tail of batch b is issued inside batch b+1.
  - Elementwise work is spread: ACT (exp, aq-even, copies), DVE (aq-odd, o4,
    reciprocals, diag reduce), GpSimd (all_reduce, diag mul, o3, broadcast).
  - C of batch b+1 is prefetched during batch b (stage pool bufs=3).
"""

import numpy as np

import concourse.bass as bass
import concourse.mybir as mybir
import concourse.tile as tile
from concourse import bacc
from concourse import bass_isa
from concourse.bass_utils import run_bass_kernel_spmd
from concourse.masks import make_identity

F32 = mybir.dt.float32
F32R = mybir.dt.float32r
AF = mybir.ActivationFunctionType
ALU = mybir.AluOpType
AX = mybir.AxisListType

B, T, J, D = 32, 1024, 128, 256
NCORES = 8
BPC = B // NCORES      # batches per core
NT = T // 128          # t-tiles per batch
ND = D // 128          # d-tiles
NCH = T // 512         # 512-wide chunks per batch
TPC = 4                # t-tiles per chunk

USE_F32R = True


def _rr(ap):
    """float32r view of an f32 AP (for rounded producers + matmul operands)."""
    return ap.bitcast(F32R) if USE_F32R else ap


def _bcast_row(ap_1d, nparts):
    """DRAM AP [n] -> [nparts, n] with partition stride 0 (DMA broadcast)."""
    return bass.AP(
        tensor=ap_1d.tensor, offset=ap_1d.offset, ap=[[0, nparts]] + list(ap_1d.ap)
    )


def _fbcast(ap_col, n):
    """[128,1] column AP -> [128, n] with free stride 0."""
    return bass.AP(
        tensor=ap_col.tensor, offset=ap_col.offset,
        ap=[list(ap_col.ap)[0], [0, n]],
    )


def _tile_bcast(ap_2d, reps):
    """[128, n] AP -> [128, reps, n] with 0-stride middle dim."""
    a = list(ap_2d.ap)
    return bass.AP(
        tensor=ap_2d.tensor, offset=ap_2d.offset,
        ap=[a[0], [0, reps]] + a[1:],
    )


def build_nc(use_f32r=None):
    global USE_F32R
    if use_f32r is not None:
        USE_F32R = use_f32r
    nc = bacc.Bacc()
    ctx_in = nc.declare_dram_parameter("context", [BPC, T, D], F32, isOutput=False)
    qry_in = nc.declare_dram_parameter("query", [BPC, J, D], F32, isOutput=False)
    w_in = nc.declare_dram_parameter("w", [3 * D], F32, isOutput=False)
    out_ext = nc.declare_dram_parameter("out", [BPC, T, 4 * D], F32, isOutput=True)

    with tile.TileContext(nc) as tc:
        _body(tc, ctx_in, qry_in, w_in, out_ext)
    nc.finalize()
    return nc


def _body(tc, ctx_in, qry_in, w_in, out_ext):
    nc = tc.nc
    from contextlib import ExitStack

    with ExitStack() as ctx:
        consts = ctx.enter_context(tc.tile_pool(name="consts", bufs=1))
        stage_p = ctx.enter_context(tc.tile_pool(name="stage", bufs=4))
        big = ctx.enter_context(tc.tile_pool(name="big", bufs=2))
        work = ctx.enter_context(tc.tile_pool(name="work", bufs=2))
        tmp = ctx.enter_context(tc.tile_pool(name="tmp", bufs=1))
        # PSUM budget (8 banks): tr 2 + g 2 + u 3 + ac 1 = 8
        ps_tr = ctx.enter_context(tc.tile_pool(name="ps_tr", bufs=2, space="PSUM"))
        ps_g = ctx.enter_context(tc.tile_pool(name="ps_g", bufs=2, space="PSUM"))
        ps_u = ctx.enter_context(tc.tile_pool(name="ps_u", bufs=3, space="PSUM"))
        ps_ac = ctx.enter_context(tc.tile_pool(name="ps_ac", bufs=1, space="PSUM"))

        loads = {}

        def load_batch(bb, nsplit):
            st = stage_p.tile([128, NT, 4 * D], F32, tag="stage")
            qt = work.tile([128, D], F32, tag="Q")
            # Q first: it gates the first PE work (Q^T transposes)
            nc.sync.dma_start(out=qt, in_=qry_in[bb])
            src = ctx_in[bb].rearrange("(p i) d -> p i d", i=NT)
            step = NT // nsplit
            for s in range(nsplit):
                nc.sync.dma_start(
                    out=st[:, s * step : (s + 1) * step, 0:D],
                    in_=src[:, s * step : (s + 1) * step, :],
                )
            loads[bb] = (st, qt)

        # batch-0 input DMAs dispatched before all the consts traffic (v7)
        load_batch(0, 4)

        # --- constants (identity first: it gates the first PE transposes) ---
        ident = consts.tile([128, 128], F32)
        make_identity(nc, ident)
        ident4 = consts.tile([128, TPC, 128], F32)
        for j in range(TPC):
            nc.scalar.copy(ident4[:, j, :], ident)
        ones_col = consts.tile([128, 1], F32)
        nc.vector.memset(ones_col, 1.0)

        # w_c / w_m as per-partition columns (two d-tiles each)
        wc_raw = consts.tile([128, ND], F32)
        wm_cols = consts.tile([128, ND], F32)
        for k in range(ND):
            nc.gpsimd.dma_start(
                out=wc_raw[:, k : k + 1],
                in_=w_in[k * 128 : (k + 1) * 128].rearrange("(p o) -> p o", o=1),
            )
            nc.gpsimd.dma_start(
                out=wm_cols[:, k : k + 1],
                in_=w_in[2 * D + k * 128 : 2 * D + (k + 1) * 128].rearrange(
                    "(p o) -> p o", o=1
                ),
            )
        # wc replicated along free dim: (wc x ones) as a rank-1 stationary
        # adds s_c[t] to every j row inside the g accumulation (v11)
        wc_rep = consts.tile([128, ND, 128], F32)
        for k in range(ND):
            nc.scalar.copy(
                _rr(wc_rep[:, k, :]), _fbcast(wc_raw[:, k : k + 1], 128)
            )
        # w_q broadcast to all partitions (for s_q = rowsum(Q * w_q))
        wq_b = consts.tile([128, D], F32)
        nc.gpsimd.dma_start(out=wq_b, in_=_bcast_row(w_in[D : 2 * D], 128))

        def beta_tail_head(S):
            """Deferred AC matmuls + s_tot -> ac_row -> broadcast for batch
            S, issued after the NEXT batch's transposes so the n_all chain
            has a full block of PE work as cover (v7)."""
            b, st, n_all, Cr = S
            ac_ps = ps_ac.tile([1, D + 2], F32, tag="ac")
            for ii in range(NT):
                nc.tensor.matmul(
                    ac_ps,
                    lhsT=_rr(n_all[:, ii : ii + 1]),
                    rhs=_rr(Cr[:, ii, :]),
                    start=(ii == 0),
                    stop=(ii == NT - 1),
                )
            r_s = work.tile([1, 1], F32, tag="r_s")
            nc.vector.reciprocal(r_s, ac_ps[0:1, D : D + 1])
            ac_row = work.tile([1, D], F32, tag="ac_row")
            nc.scalar.activation(ac_row, ac_ps[0:1, 0:D], AF.Copy, scale=r_s)
            acb = work.tile([128, D], F32, tag="acb")
            nc.gpsimd.partition_broadcast(acb, ac_row, channels=128)
            return acb

        def beta_tail_finish(S, acb, nsplit=2):
            """o4 = C * AC (DVE) + staged output DMA, split so the first
            part's DMA fires while later parts compute (v8/v12)."""
            b, st = S[0], S[1]
            out_r = out_ext[b].rearrange("(p i) d -> p i d", i=NT)
            h = NT // nsplit
            for s in range(nsplit):
                sl = slice(s * h, (s + 1) * h)
                nc.vector.tensor_mul(
                    st[:, sl, 3 * D : 4 * D], st[:, sl, 0:D], _tile_bcast(acb, h)
                )
                nc.sync.dma_start(out=out_r[:, sl, :], in_=st[:, sl, :])

        prev = None
        for b in range(BPC):
            if b + 1 < BPC:
                load_batch(b + 1, 2)
            st, Q = loads.pop(b)

            # ---- Q^T, with w_m folded in: qwT[d, j] = Q[j, d] * w_m[d] ----
            qt_ps = ps_tr.tile([128, D], F32, tag="tr")
            for k in range(ND):
                nc.tensor.transpose(
                    qt_ps[:, k * 128 : (k + 1) * 128], Q[:, k * 128 : (k + 1) * 128],
                    ident,
                )
            qwT = work.tile([128, D], F32, tag="qwT")
            for k in range(ND):
                nc.scalar.activation(
                    _rr(qwT[:, k * 128 : (k + 1) * 128]),
                    qt_ps[:, k * 128 : (k + 1) * 128],
                    AF.Copy,
                    scale=wm_cols[:, k : k + 1],
                )

            # ---- Qaug = [Q | 1] rounded (rhs of the U matmuls) ----
            Qaug = work.tile([128, D + 2], F32, tag="Qaug")
            nc.scalar.copy(_rr(Qaug[:, 0:D]), Q)
            nc.vector.tensor_copy(
                _rr(Qaug[:, D : D + 2]), _fbcast(ones_col, 2)
            )

            # ---- s_q column: rowsum(Q * w_q) ----
            sq_scr = tmp.tile([128, D], F32, tag="sq_scr")
            sq_col = work.tile([128, 1], F32, tag="sq_col")
            nc.vector.tensor_mul(sq_scr, Q, wq_b)
            nc.vector.reduce_sum(out=sq_col, in_=sq_scr, axis=AX.X)

            # ---- C^T via PE transposes ----
            ctT = big.tile([128, ND, NT, 128], F32, tag="ctT")
            for i2 in range(NT // 2):
                ct_ps = ps_tr.tile([128, 2 * ND * 128], F32, tag="tr")
                for u in range(2):
                    i = 2 * i2 + u
                    for k in range(ND):
                        nc.tensor.transpose(
                            ct_ps[:, (2 * u + k) * 128 : (2 * u + k + 1) * 128],
                            st[:, i, k * 128 : (k + 1) * 128],
                            ident,
                        )
                dst = _rr(ctT[:, :, 2 * i2 : 2 * i2 + 2, :])
                srcv = ct_ps.rearrange("p (t k x) -> p k t x", t=2, k=ND)
                if i2 % 2 == 0:
                    nc.scalar.copy(dst, srcv)
                else:
                    nc.vector.tensor_copy(dst, srcv)

            # ---- previous batch: deferred AC + s_tot/ac_row/bcast (v7) ----
            acb_prev = beta_tail_head(prev) if prev is not None else None

            # ---- per 512-chunk: g + s_c row matmuls ----
            ET = big.tile([128, T], F32, tag="ET")
            n_all = work.tile([128, NT], F32, tag="n_all")
            g_list = []
            for c in range(NCH):
                g_ps = ps_g.tile([128, 512], F32, tag="g")
                for k in range(ND):
                    nc.tensor.matmul(
                        g_ps,
                        lhsT=_rr(qwT[:, k * 128 : (k + 1) * 128]),
                        rhs=_rr(ctT[:, k, TPC * c : TPC * (c + 1), :]),
                        start=(k == 0),
                        stop=False,
                        skip_group_check=True,
                    )
                for k in range(ND):
                    nc.tensor.matmul(
                        g_ps,
                        lhsT=_rr(wc_rep[:, k, :]),
                        rhs=_rr(ctT[:, k, TPC * c : TPC * (c + 1), :]),
                        start=False,
                        stop=(k == ND - 1),
                        skip_group_check=True,
                    )
                g_list.append(g_ps)

            # ---- Cr = [C | 1] rounded (rhs of the AC matmuls) ----
            Cr = big.tile([128, NT, D + 2], F32, tag="Cr")
            nc.vector.tensor_copy(
                _rr(Cr[:, :, D : D + 2]),
                bass.AP(tensor=ones_col.tensor, offset=ones_col.offset,
                        ap=[list(ones_col.ap)[0], [0, NT], [0, 2]]),
            )
            h = NT // 2
            nc.scalar.copy(_rr(Cr[:, 0:h, 0:D]), st[:, 0:h, 0:D])
            nc.vector.tensor_copy(_rr(Cr[:, h:NT, 0:D]), st[:, h:NT, 0:D])

            # ---- per chunk: fold s_c (rank-1), E' = exp, colmax, diag ----
            # o4 + out-DMA of the previous batch are issued between the two
            # chunks so the DVE has work while the GpSimd all_reduce runs.
            me_list = []
            for c in range(NCH):
                g_ps = g_list[c]
                nc.scalar.activation(
                    _rr(ET[:, c * 512 : (c + 1) * 512]), g_ps, AF.Exp, bias=sq_col
                )
                me = tmp.tile([128, 512], F32, tag=f"me{c}")
                # the tail chunk's reduce goes in halves so the final beta
                # chain is ~1.3us shorter on the last batch (v8)
                nh = 2 if (b == BPC - 1 and c == NCH - 1) else 1
                for hh in range(nh):
                    w512 = 512 // nh
                    nc.gpsimd.partition_all_reduce(
                        me[:, hh * w512 : (hh + 1) * w512],
                        ET[:, c * 512 + hh * w512 : c * 512 + (hh + 1) * w512],
                        channels=128,
                        reduce_op=bass_isa.ReduceOp.max,
                    )
                me_list.append((me, nh))
            for c in range(NCH):
                me, nh = me_list[c]
                scr = tmp.tile([128, TPC, 128], F32, tag=f"scr{c}")
                mev = me.rearrange("p (i r) -> p i r", r=128)
                for hh in range(nh):
                    tp = TPC // nh
                    sl = slice(hh * tp, (hh + 1) * tp)
                    nc.vector.tensor_mul(scr[:, sl, :], mev[:, sl, :], ident4[:, 0:tp, :])
                    nc.vector.reduce_max(
                        out=_rr(n_all[:, TPC * c + hh * tp : TPC * c + (hh + 1) * tp]),
                        in_=scr[:, sl, :],
                        axis=AX.X,
                    )

            # ---- per t-tile: U = E' @ [Q|1]; aq on ACT; o3 on DVE ----
            r_all = work.tile([128, NT], F32, tag="r_all")
            for i in range(NT):
                u_ps = ps_u.tile([128, D + 2], F32, tag="u")
                nc.tensor.matmul(
                    u_ps, lhsT=_rr(ET[:, i * 128 : (i + 1) * 128]), rhs=_rr(Qaug)
                )
                nc.vector.reciprocal(r_all[:, i : i + 1], u_ps[:, D : D + 1])
                nc.scalar.activation(
                    st[:, i, D : 2 * D], u_ps[:, 0:D], AF.Copy,
                    scale=r_all[:, i : i + 1],
                )
                if i % TPC == TPC - 1:
                    # o3 for this chunk's 4 tiles (one DVE pass)
                    j0 = i - (TPC - 1)
                    nc.vector.tensor_mul(
                        st[:, j0 : i + 1, 2 * D : 3 * D],
                        st[:, j0 : i + 1, D : 2 * D],
                        st[:, j0 : i + 1, 0:D],
                    )

            # ---- previous batch: o4 + output DMA (after the U loop so the
            #      DVE recips/aq aren't stuck behind the big o4 pass) ----
            if prev is not None:
                beta_tail_finish(prev, acb_prev)
            prev = (b, st, n_all, Cr)

        beta_tail_finish(prev, beta_tail_head(prev), nsplit=4)


_NC_CACHE = {}


def kernel(context, query, w, b, _trace=False):
    context = np.ascontiguousarray(context, dtype=np.float32)
    query = np.ascontiguousarray(query, dtype=np.float32)
    w = np.ascontiguousarray(w, dtype=np.float32)

    if "nc" not in _NC_CACHE:
        _NC_CACHE["nc"] = build_nc()
    nc = _NC_CACHE["nc"]

    in_maps = [
        {
            "context": context[i * BPC : (i + 1) * BPC],
            "query": query[i * BPC : (i + 1) * BPC],
            "w": w,
        }
        for i in range(NCORES)
    ]
    try:
        res = run_bass_kernel_spmd(
            nc, in_maps, core_ids=list(range(NCORES)), trace=_trace
        )
    except Exception:
        # A previous process may have left the device wedged; reset and retry.
        import ctypes

        import jax

        jax.devices()
        lib = ctypes.CDLL("/opt/axon/libaxon_pjrt.so")
        if hasattr(lib, "axon_reset"):
            lib.axon_reset()
        res = run_bass_kernel_spmd(
            nc, in_maps, core_ids=list(range(NCORES)), trace=_trace
        )
    out = np.concatenate([res.results[i]["out"] for i in range(NCORES)], axis=0)
    if _trace:
        kernel.last_exec_time_ns = res.exec_time_ns
        kernel.last_results = res
    return out


if __name__ == "__main__":
    rng = np.random.default_rng(0)
    inputs = {
        "context": rng.standard_normal((B, T, D), dtype=np.float32),
        "query": rng.standard_normal((B, J, D), dtype=np.float32),
        "w": (rng.standard_normal(3 * D).astype(np.float32) / np.sqrt(3 * D)),
        "b": np.zeros(1, np.float32),
    }
    out = kernel(**inputs)
    print("out", out.shape, out.dtype, float(np.abs(out).mean()))


# revision 20
# speedup vs baseline: 1.3543x; 1.1256x over previous
"""AttentionFlow Trainium2 Bass kernel (v2).

Math (per batch):
  d = 256; w = [w_c | w_q | w_m]
  sim[t,j] = s_c[t] + s_q[j] + sum_d C[t,d] w_m[d] Q[j,d]   (+b, which cancels)
  attn = softmax_j(sim);  AQ = attn @ Q
  beta = softmax_t(max_j sim);  AC = beta @ C
  out = concat([C, AQ, C*AQ, C*AC], axis=-1)

Sharding: data-parallel over batch B=32 across 8 NeuronCores (4 batches/core).

Design (~90us, vs ~95-108us for the v1 baseline):
  - Permuted t-layout t = 8p + i (partition-major): each batch's whole output
    row-block is contiguous per partition in HBM, so output DMAs use 4KB+
    lines (32KB/partition) instead of ~3k 1-2KB lines.  All per-t math is
    permutation-invariant; only the T-sums (AC, s_tot) mix t and they are
    order-free.
  - Everything is staged in one [128, NT, 4D] tile per batch
    (cols 0:256 = C landed directly by the input DMA, 256:512 aq,
    512:768 o3=C*aq, 768:1024 o4=C*AC); stage pool bufs=4 so C of batch b+1
    prefetches without waiting on out-DMA completions.  Q is dispatched
    before C: it gates the first PE work of a batch.
  - E' = exp(g + s_q + s_c) holds the FULL similarity: s_q enters as the
    ACT bias column of the exp; s_c enters INSIDE the g PSUM accumulation
    via two extra matmuls with rank-1 stationaries (wc_k x ones) streaming
    the same ctT chunks.  Then n[t] = colmax_j E' directly (no exp(s_c)
    row pass, no n-row muls, no N=1 transposes), and attn normalization is
    unchanged (exp(s_c[t]) cancels in U[:,0:256]/U[:,256]).
  - n columns come from GpSimd partition_all_reduce(max) + a diagonal
    extract (DVE mul by identity + reduce_max).  GpSimd runs ONLY
    partition_all_reduce / partition_broadcast: mixing in gpsimd tensor ops
    forces ucode library swaps at ~6-7us each.
  - U = E' @ [Q | 1 | 1] as ONE N=258 fp32r matmul per t-tile (fp32r needs
    even moving/output sizes, hence the doubled ones column); AC = n^T @
    [C | 1 | 1] likewise, so s_tot falls out of the same accumulation.
  - The beta tail of batch b (AC matmuls, s_tot, broadcast, o4, output DMA)
    is deferred into batch b+1's body after its transposes, so the
    exp -> all_reduce -> diag chain (~5us) is covered by independent PE
    work; PE idle gaps > 3.4us would otherwise also re-engage the HAM
    half-clock throttle.  The last batch splits its tail all_reduce into
    halves and its o4+DMA into quarters to shorten the exposed chain.
"""

import numpy as np

import concourse.bass as bass
import concourse.mybir as mybir
import concourse.tile as tile
from concourse import bacc
from concourse import bass_isa
from concourse.bass_utils import run_bass_kernel_spmd
from concourse.masks import make_identity

F32 = mybir.dt.float32
F32R = mybir.dt.float32r
AF = mybir.ActivationFunctionType
ALU = mybir.AluOpType
AX = mybir.AxisListType

B, T, J, D = 32, 1024, 128, 256
NCORES = 8
BPC = B // NCORES      # batches per core
NT = T // 128          # t-tiles per batch
ND = D // 128          # d-tiles
NCH = T // 512         # 512-wide chunks per batch
TPC = 4                # t-tiles per chunk

USE_F32R = True


def _rr(ap):
    """float32r view of an f32 AP (for rounded producers + matmul operands)."""
    return ap.bitcast(F32R) if USE_F32R else ap


def _bcast_row(ap_1d, nparts):
    """DRAM AP [n] -> [nparts, n] with partition stride 0 (DMA broadcast)."""
    return bass.AP(
        tensor=ap_1d.tensor, offset=ap_1d.offset, ap=[[0, nparts]] + list(ap_1d.ap)
    )


def _fbcast(ap_col, n):
    """[128,1] column AP -> [128, n] with free stride 0."""
    return bass.AP(
        tensor=ap_col.tensor, offset=ap_col.offset,
        ap=[list(ap_col.ap)[0], [0, n]],
    )


def _tile_bcast(ap_2d, reps):
    """[128, n] AP -> [128, reps, n] with 0-stride middle dim."""
    a = list(ap_2d.ap)
    return bass.AP(
        tensor=ap_2d.tensor, offset=ap_2d.offset,
        ap=[a[0], [0, reps]] + a[1:],
    )


def build_nc(use_f32r=None):
    global USE_F32R
    if use_f32r is not None:
        USE_F32R = use_f32r
    nc = bacc.Bacc()
    ctx_in = nc.declare_dram_parameter("context", [BPC, T, D], F32, isOutput=False)
    qry_in = nc.declare_dram_parameter("query", [BPC, J, D], F32, isOutput=False)
    w_in = nc.declare_dram_parameter("w", [3 * D], F32, isOutput=False)
    out_ext = nc.declare_dram_parameter("out", [BPC, T, 4 * D], F32, isOutput=True)

    with tile.TileContext(nc) as tc:
        _body(tc, ctx_in, qry_in, w_in, out_ext)
    nc.finalize()
    return nc


def _body(tc, ctx_in, qry_in, w_in, out_ext):
    nc = tc.nc
    from contextlib import ExitStack

    with ExitStack() as ctx:
        consts = ctx.enter_context(tc.tile_pool(name="consts", bufs=1))
        stage_p = ctx.enter_context(tc.tile_pool(name="stage", bufs=4))
        big = ctx.enter_context(tc.tile_pool(name="big", bufs=2))
        work = ctx.enter_context(tc.tile_pool(name="work", bufs=2))
        tmp = ctx.enter_context(tc.tile_pool(name="tmp", bufs=1))
        # PSUM budget (8 banks): tr 2 + g 2 + u 3 + ac 1 = 8
        ps_tr = ctx.enter_context(tc.tile_pool(name="ps_tr", bufs=2, space="PSUM"))
        ps_g = ctx.enter_context(tc.tile_pool(name="ps_g", bufs=2, space="PSUM"))
        ps_u = ctx.enter_context(tc.tile_pool(name="ps_u", bufs=3, space="PSUM"))
        ps_ac = ctx.enter_context(tc.tile_pool(name="ps_ac", bufs=1, space="PSUM"))

        loads = {}

        def load_batch(bb, nsplit):
            st = stage_p.tile([128, NT, 4 * D], F32, tag="stage")
            qt = work.tile([128, D], F32, tag="Q")
            # Q first: it gates the first PE work (Q^T transposes)
            nc.sync.dma_start(out=qt, in_=qry_in[bb])
            src = ctx_in[bb].rearrange("(p i) d -> p i d", i=NT)
            step = NT // nsplit
            for s in range(nsplit):
                nc.sync.dma_start(
                    out=st[:, s * step : (s + 1) * step, 0:D],
                    in_=src[:, s * step : (s + 1) * step, :],
                )
            loads[bb] = (st, qt)

        # batch-0 input DMAs dispatched before all the consts traffic (v7)
        load_batch(0, 4)

        # --- constants (identity first: it gates the first PE transposes) ---
        ident = consts.tile([128, 128], F32)
        make_identity(nc, ident)
        ones_col = consts.tile([128, 1], F32)
        nc.vector.memset(ones_col, 1.0)

        # w_c / w_m as per-partition columns (two d-tiles each)
        wc_raw = consts.tile([128, ND], F32)
        wm_cols = consts.tile([128, ND], F32)
        for k in range(ND):
            nc.gpsimd.dma_start(
                out=wc_raw[:, k : k + 1],
                in_=w_in[k * 128 : (k + 1) * 128].rearrange("(p o) -> p o", o=1),
            )
            nc.gpsimd.dma_start(
                out=wm_cols[:, k : k + 1],
                in_=w_in[2 * D + k * 128 : 2 * D + (k + 1) * 128].rearrange(
                    "(p o) -> p o", o=1
                ),
            )
        # wc replicated along free dim: (wc x ones) as a rank-1 stationary
        # adds s_c[t] to every j row inside the g accumulation (v11)
        wc_rep = consts.tile([128, ND, 128], F32)
        for k in range(ND):
            nc.scalar.copy(
                _rr(wc_rep[:, k, :]), _fbcast(wc_raw[:, k : k + 1], 128)
            )
        # w_q broadcast to all partitions (for s_q = rowsum(Q * w_q))
        wq_b = consts.tile([128, D], F32)
        nc.gpsimd.dma_start(out=wq_b, in_=_bcast_row(w_in[D : 2 * D], 128))

        def beta_tail_head(S):
            """Deferred AC matmuls + s_tot -> ac_row -> broadcast for batch
            S, issued after the NEXT batch's transposes so the n_all chain
            has a full block of PE work as cover (v7)."""
            b, st, n_all, Cr = S
            ac_ps = ps_ac.tile([1, D + 2], F32, tag="ac")
            for ii in range(NT):
                nc.tensor.matmul(
                    ac_ps,
                    lhsT=_rr(n_all[:, ii : ii + 1]),
                    rhs=_rr(Cr[:, ii, :]),
                    start=(ii == 0),
                    stop=(ii == NT - 1),
                )
            r_s = work.tile([1, 1], F32, tag="r_s")
            nc.vector.reciprocal(r_s, ac_ps[0:1, D : D + 1])
            ac_row = work.tile([1, D], F32, tag="ac_row")
            nc.scalar.activation(ac_row, ac_ps[0:1, 0:D], AF.Copy, scale=r_s)
            acb = work.tile([128, D], F32, tag="acb")
            nc.gpsimd.partition_broadcast(acb, ac_row, channels=128)
            return acb

        def beta_tail_finish(S, acb, nsplit=2):
            """o4 = C * AC (DVE) + staged output DMA, split so the first
            part's DMA fires while later parts compute (v8/v12)."""
            b, st = S[0], S[1]
            out_r = out_ext[b].rearrange("(p i) d -> p i d", i=NT)
            h = NT // nsplit
            for s in range(nsplit):
                sl = slice(s * h, (s + 1) * h)
                nc.vector.tensor_mul(
                    st[:, sl, 3 * D : 4 * D], st[:, sl, 0:D], _tile_bcast(acb, h)
                )
                nc.sync.dma_start(out=out_r[:, sl, :], in_=st[:, sl, :])

        prev = None
        for b in range(BPC):
            if b + 1 < BPC:
                load_batch(b + 1, 2)
            st, Q = loads.pop(b)

            # ---- Q^T, with w_m folded in: qwT[d, j] = Q[j, d] * w_m[d] ----
            qt_ps = ps_tr.tile([128, D], F32, tag="tr")
            for k in range(ND):
                nc.tensor.transpose(
                    qt_ps[:, k * 128 : (k + 1) * 128], Q[:, k * 128 : (k + 1) * 128],
                    ident,
                )
            qwT = work.tile([128, D], F32, tag="qwT")
            for k in range(ND):
                nc.scalar.activation(
                    _rr(qwT[:, k * 128 : (k + 1) * 128]),
                    qt_ps[:, k * 128 : (k + 1) * 128],
                    AF.Copy,
                    scale=wm_cols[:, k : k + 1],
                )

            # ---- Qaug = [Q | 1] rounded (rhs of the U matmuls) ----
            Qaug = work.tile([128, D + 2], F32, tag="Qaug")
            nc.scalar.copy(_rr(Qaug[:, 0:D]), Q)
            nc.vector.tensor_copy(
                _rr(Qaug[:, D : D + 2]), _fbcast(ones_col, 2)
            )

            # ---- s_q column: rowsum(Q * w_q) ----
            sq_scr = tmp.tile([128, D], F32, tag="sq_scr")
            sq_col = work.tile([128, 1], F32, tag="sq_col")
            nc.vector.tensor_mul(sq_scr, Q, wq_b)
            nc.vector.reduce_sum(out=sq_col, in_=sq_scr, axis=AX.X)

            # ---- C^T via PE transposes ----
            ctT = big.tile([128, ND, NT, 128], F32, tag="ctT")
            for i2 in range(NT // 2):
                ct_ps = ps_tr.tile([128, 2 * ND * 128], F32, tag="tr")
                for u in range(2):
                    i = 2 * i2 + u
                    for k in range(ND):
                        nc.tensor.transpose(
                            ct_ps[:, (2 * u + k) * 128 : (2 * u + k + 1) * 128],
                            st[:, i, k * 128 : (k + 1) * 128],
                            ident,
                        )
                dst = _rr(ctT[:, :, 2 * i2 : 2 * i2 + 2, :])
                srcv = ct_ps.rearrange("p (t k x) -> p k t x", t=2, k=ND)
                if i2 % 2 == 0:
                    nc.scalar.copy(dst, srcv)
                else:
                    nc.vector.tensor_copy(dst, srcv)

            # ---- previous batch: deferred AC + s_tot/ac_row/bcast (v7) ----
            acb_prev = beta_tail_head(prev) if prev is not None else None

            # ---- per 512-chunk: g + s_c row matmuls ----
            ET = big.tile([128, T], F32, tag="ET")
            n_all = work.tile([128, NT], F32, tag="n_all")
            g_list = []
            for c in range(NCH):
                g_ps = ps_g.tile([128, 512], F32, tag="g")
                for k in range(ND):
                    nc.tensor.matmul(
                        g_ps,
                        lhsT=_rr(qwT[:, k * 128 : (k + 1) * 128]),
                        rhs=_rr(ctT[:, k, TPC * c : TPC * (c + 1), :]),
                        start=(k == 0),
                        stop=False,
                        skip_group_check=True,
                    )
                for k in range(ND):
                    nc.tensor.matmul(
                        g_ps,
                        lhsT=_rr(wc_rep[:, k, :]),
                        rhs=_rr(ctT[:, k, TPC * c : TPC * (c + 1), :]),
                        start=False,
                        stop=(k == ND - 1),
                        skip_group_check=True,
                    )
                g_list.append(g_ps)

            # ---- Cr = [C | 1] rounded (rhs of the AC matmuls) ----
            Cr = big.tile([128, NT, D + 2], F32, tag="Cr")
            nc.vector.tensor_copy(
                _rr(Cr[:, :, D : D + 2]),
                bass.AP(tensor=ones_col.tensor, offset=ones_col.offset,
                        ap=[list(ones_col.ap)[0], [0, NT], [0, 2]]),
            )
            h = NT // 2
            nc.scalar.copy(_rr(Cr[:, 0:h, 0:D]), st[:, 0:h, 0:D])
            nc.vector.tensor_copy(_rr(Cr[:, h:NT, 0:D]), st[:, h:NT, 0:D])

            # ---- per chunk: fold s_c (rank-1), E' = exp, colmax, diag ----
            # o4 + out-DMA of the previous batch are issued between the two
            # chunks so the DVE has work while the GpSimd all_reduce runs.
            for c in range(NCH):
                g_ps = g_list[c]
                nc.scalar.activation(
                    _rr(ET[:, c * 512 : (c + 1) * 512]), g_ps, AF.Exp, bias=sq_col
                )
                # colmax over j via PE transposes of E' tiles + one DVE
                # free-dim reduce_max -- keeps GpSimd off the critical path
                # (its all_reduce ran ~2us per chunk and serialized) (v13)
                tp_ps = ps_tr.tile([128, TPC, 128], F32, tag="tr")
                for i in range(TPC):
                    t0 = (TPC * c + i) * 128
                    nc.tensor.transpose(
                        tp_ps[:, i, :], ET[:, t0 : t0 + 128], ident
                    )
                nc.vector.reduce_max(
                    out=_rr(n_all[:, TPC * c : TPC * (c + 1)]), in_=tp_ps,
                    axis=AX.X,
                )

            # ---- per t-tile: U = E' @ [Q|1]; aq on ACT; o3 on DVE ----
            r_all = work.tile([128, NT], F32, tag="r_all")
            for i in range(NT):
                u_ps = ps_u.tile([128, D + 2], F32, tag="u")
                nc.tensor.matmul(
                    u_ps, lhsT=_rr(ET[:, i * 128 : (i + 1) * 128]), rhs=_rr(Qaug)
                )
                nc.vector.reciprocal(r_all[:, i : i + 1], u_ps[:, D : D + 1])
                nc.scalar.activation(
                    st[:, i, D : 2 * D], u_ps[:, 0:D], AF.Copy,
                    scale=r_all[:, i : i + 1],
                )
                if i % TPC == TPC - 1:
                    # o3 for this chunk's 4 tiles (one DVE pass)
                    j0 = i - (TPC - 1)
                    nc.vector.tensor_mul(
                        st[:, j0 : i + 1, 2 * D : 3 * D],
                        st[:, j0 : i + 1, D : 2 * D],
                        st[:, j0 : i + 1, 0:D],
                    )

            # ---- previous batch: o4 + output DMA (after the U loop so the
            #      DVE recips/aq aren't stuck behind the big o4 pass) ----
            if prev is not None:
                beta_tail_finish(prev, acb_prev)
            prev = (b, st, n_all, Cr)

        beta_tail_finish(prev, beta_tail_head(prev), nsplit=4)


_NC_CACHE = {}


def kernel(context, query, w, b, _trace=False):
    context = np.ascontiguousarray(context, dtype=np.float32)
    query = np.ascontiguousarray(query, dtype=np.float32)
    w = np.ascontiguousarray(w, dtype=np.float32)

    if "nc" not in _NC_CACHE:
        _NC_CACHE["nc"] = build_nc()
    nc = _NC_CACHE["nc"]

    in_maps = [
        {
            "context": context[i * BPC : (i + 1) * BPC],
            "query": query[i * BPC : (i + 1) * BPC],
            "w": w,
        }
        for i in range(NCORES)
    ]
    try:
        res = run_bass_kernel_spmd(
            nc, in_maps, core_ids=list(range(NCORES)), trace=_trace
        )
    except Exception:
        # A previous process may have left the device wedged; reset and retry.
        import ctypes

        import jax

        jax.devices()
        lib = ctypes.CDLL("/opt/axon/libaxon_pjrt.so")
        if hasattr(lib, "axon_reset"):
            lib.axon_reset()
        res = run_bass_kernel_spmd(
            nc, in_maps, core_ids=list(range(NCORES)), trace=_trace
        )
    out = np.concatenate([res.results[i]["out"] for i in range(NCORES)], axis=0)
    if _trace:
        kernel.last_exec_time_ns = res.exec_time_ns
        kernel.last_results = res
    return out


if __name__ == "__main__":
    rng = np.random.default_rng(0)
    inputs = {
        "context": rng.standard_normal((B, T, D), dtype=np.float32),
        "query": rng.standard_normal((B, J, D), dtype=np.float32),
        "w": (rng.standard_normal(3 * D).astype(np.float32) / np.sqrt(3 * D)),
        "b": np.zeros(1, np.float32),
    }
    out = kernel(**inputs)
    print("out", out.shape, out.dtype, float(np.abs(out).mean()))
